# revision 1
# baseline (speedup 1.0000x reference)
"""EvolvedAttention Trainium2 Bass kernel.

Full inputs -> full output. Sharding: 8 cores = 2 batches x 4 query-row
slices. Each core computes K/V/attention for its (batch, row-slice) with
all 16 heads; host only slices inputs and concatenates row-slice outputs.

Per-core pipeline:
  - projections in fp32r (near-fp32, full PE rate at N>=256)
  - cosine normalization per-partition in row-major layouts; Q/K transposed
    to head-major [dh, seq] via PE transposes (fp16)
  - per-row top-k threshold (k = S/4) by counting passes
    (tensor_scalar is_ge + accum) with bracketed false-position updates
  - scores recomputed transposed with the threshold folded in as a rank-1
    term (ones row in Kn, -t/tau row in Qn, contraction K=65)
  - exp on ScalarE from PSUM -> fp16 E, mask E*[E>=1] (stt), AV matmul with
    a ones column for the softmax denominator
  - output projection + sigmoid highway gate on device

SBUF is phased with nested tile pools; Kn^T and the gate are staged
through DRAM to keep the working set under the SBUF limit.
"""

import os
import numpy as np

import concourse.bass as bass
import concourse.mybir as mybir
import concourse.tile as tile
from concourse import bacc

FP32 = mybir.dt.float32
FP32R = mybir.dt.float32r
FP16 = mybir.dt.float16
U8 = mybir.dt.uint8
AF = mybir.ActivationFunctionType
ALU = mybir.AluOpType


class Cfg:
    def __init__(self, S=2048, D=1024, NH=16, RS=512, n_sel_iters=4):
        self.S = S
        self.D = D
        self.NH = NH
        self.DH = D // NH
        self.RS = RS
        self.KK = S // 4
        self.DCH = D // 128
        self.KC = S // 128
        self.RC = RS // 128
        self.NW = min(512, D)
        self.ND = D // self.NW
        self.KW = min(512, S)
        self.NKC = S // self.KW
        self.HP = NH // 2
        self.GROUP = 4 if NH % 4 == 0 else NH
        self.n_sel_iters = n_sel_iters
        self.slope0 = 2.8 * S


def build(cfg: Cfg):
    nc = bacc.Bacc()
    S, D, NH, DH, RS = cfg.S, cfg.D, cfg.NH, cfg.DH, cfg.RS
    DCH, KC, RC, HP, NW, KW = cfg.DCH, cfg.KC, cfg.RC, cfg.HP, cfg.NW, cfg.KW

    xT = nc.dram_tensor("xT", [128, DCH, S], FP32R, kind="ExternalInput")
    xs = nc.dram_tensor("xs", [RS, D], FP32, kind="ExternalInput")
    Wq = nc.dram_tensor("Wq", [128, DCH, D], FP32R, kind="ExternalInput")
    Wk = nc.dram_tensor("Wk", [128, DCH, D], FP32R, kind="ExternalInput")
    Wv = nc.dram_tensor("Wv", [128, DCH, D], FP32R, kind="ExternalInput")
    Wg = nc.dram_tensor("Wg", [128, DCH, D], FP32R, kind="ExternalInput")
    Wo = nc.dram_tensor("Wo", [128, HP, D], FP16, kind="ExternalInput")
    Wt = nc.dram_tensor("Wt", [128, DCH], FP32R, kind="ExternalInput")
    bq = nc.dram_tensor("bq", [1, D], FP32R, kind="ExternalInput")
    bk = nc.dram_tensor("bk", [1, D], FP32R, kind="ExternalInput")
    bv = nc.dram_tensor("bv", [1, D], FP32R, kind="ExternalInput")
    bg = nc.dram_tensor("bg", [1, D], FP32R, kind="ExternalInput")
    bo = nc.dram_tensor("bo", [1, D], FP16, kind="ExternalInput")
    bt = nc.dram_tensor("bt", [1, 1], FP32, kind="ExternalInput")
    out = nc.dram_tensor("out", [RS, D], FP32, kind="ExternalOutput")
    knt_dram = nc.dram_tensor("knt_dram", [HP, 128, S], FP16)
    gate_dram = nc.dram_tensor("gate_dram", [128, RC, D], FP16)

    with tile.TileContext(nc) as tc:
        with (
            tc.tile_pool(name="persist", bufs=1) as pp,
            tc.tile_pool(name="psum", bufs=2, space="PSUM") as ps,
        ):
            QnT = [pp.tile([65, RS], FP16, tag=f"qnt{h}", name=f"qnt{h}")
                   for h in range(NH)]
            V16 = pp.tile([128, KC, NH, 65], FP16, tag="v16")
            attnT = pp.tile([128, HP, RS], FP16, tag="attnT")
            ident = pp.tile([128, 128], FP16, tag="ident")
            from concourse.masks import make_identity
            make_identity(nc, ident[:])
            ones_r32 = pp.tile([1, 128], FP32, tag="ones_r")
            nc.vector.memset(ones_r32[:], 1.0)
            ones_r = ones_r32[:].bitcast(FP32R)
            ones_h = pp.tile([1, 128], FP16, tag="ones_h")
            nc.vector.memset(ones_h[:], 1.0)
            nc.vector.memset(V16[:], 1.0)
            bias_r = {}
            for nm, dram in (("bq", bq), ("bk", bk), ("bv", bv), ("bg", bg)):
                t = pp.tile([1, D], FP32R, tag=nm, name=f"b_{nm}")
                nc.sync.dma_start(t[:], dram[:])
                bias_r[nm] = t
            bo_t = pp.tile([1, D], FP16, tag="bo")
            nc.sync.dma_start(bo_t[:], bo[:])
            bt_t = pp.tile([1, 1], FP32, tag="bt")
            nc.sync.dma_start(bt_t[:], bt[:])
            wt_t = pp.tile([128, DCH], FP32R, tag="wt")
            nc.sync.dma_start(wt_t[:], Wt[:])
            invt128 = pp.tile([128, 1], FP32, tag="invt128")

            def proj_rowmajor(xt_tile, w_dram, bias_row, n_chunks, wpool):
                w = wpool.tile([128, DCH, D], FP32R, tag="wbig", name="wbig", bufs=1)
                nc.sync.dma_start(w[:], w_dram[:])
                for j in range(n_chunks):
                    pt = ps.tile([128, D], FP32, tag="projp", bufs=2,
                                 name="pt_proj")
                    for c in range(DCH):
                        for n in range(cfg.ND):
                            nc.tensor.matmul(
                                pt[:, n * NW : (n + 1) * NW],
                                xt_tile[:, c, j * 128 : (j + 1) * 128],
                                w[:, c, n * NW : (n + 1) * NW],
                                start=(c == 0), stop=False)
                    for n in range(cfg.ND):
                        nc.tensor.matmul(
                            pt[:, n * NW : (n + 1) * NW],
                            ones_r, bias_row[:, n * NW : (n + 1) * NW],
                            start=False, stop=True)
                    yield pt

            def normalize_chunk(sp, pt, dst16, extra_scale_ap):
                sq = sp.tile([128, D], FP32, tag="sq", name="sq", bufs=2)
                nc.scalar.activation(sq[:], pt[:], AF.Square)
                n2 = sp.tile([128, NH], FP32, tag="n2", name="n2", bufs=2)
                nc.vector.tensor_reduce(
                    n2[:], sq[:].rearrange("p (h d) -> p h d", h=NH),
                    axis=mybir.AxisListType.X, op=ALU.add)
                nc.vector.tensor_scalar_max(n2[:], n2[:], 1e-24)
                rec = sp.tile([128, NH], FP32, tag="rec", name="rec", bufs=2)
                nc.vector.reciprocal(rec[:], n2[:])
                rsq = sp.tile([128, NH], FP32, tag="rsq", name="rsq", bufs=2)
                nc.scalar.activation(rsq[:], rec[:], AF.Sqrt)
                if extra_scale_ap is not None:
                    nc.vector.tensor_scalar(
                        out=rsq[:], in0=rsq[:], scalar1=extra_scale_ap,
                        scalar2=None, op0=ALU.mult)
                nc.vector.tensor_tensor(
                    dst16[:].rearrange("p (h d) -> p h d", h=NH),
                    pt[:].rearrange("p (h d) -> p h d", h=NH),
                    rsq[:].rearrange("p (h o) -> p h o", o=1)
                        .to_broadcast([128, NH, DH]),
                    ALU.mult)

            def transpose_to_heads(dst_of_head, src16, j):
                for p in range(HP):
                    tps = ps.tile([128, 128], FP16, tag="p512", bufs=4,
                                  padded_shape=[128, max(KW, RS)], name="tps")
                    nc.tensor.transpose(tps[:],
                                        src16[:, p * 128 : (p + 1) * 128],
                                        ident[:])
                    for hh in range(2):
                        h = 2 * p + hh
                        dst = dst_of_head(h)[0:64, j * 128 : (j + 1) * 128]
                        src = tps[hh * 64 : hh * 64 + 64, :]
                        if (p + hh) % 2 == 0:
                            nc.scalar.activation(dst, src, AF.Copy)
                        else:
                            nc.vector.tensor_copy(dst, src)

            # ======== phase A1: temp, K, V (needs full xT) ========
            with (
                tc.tile_pool(name="poolA1", bufs=1) as pa,
                tc.tile_pool(name="wpoolA1", bufs=2) as wpa,
            ):
                xt = pa.tile([128, DCH, S], FP32R, tag="xt")
                nc.sync.dma_start(xt[:], xT[:])

                tp = ps.tile([1, KW], FP32, tag="p512", bufs=4,
                             padded_shape=[128, max(KW, RS)], name="tp_temp")
                first = True
                for c in range(DCH):
                    for j in range(cfg.NKC):
                        nc.tensor.matmul(
                            tp[:], wt_t[:, c : c + 1],
                            xt[:, c, j * KW : (j + 1) * KW],
                            start=first,
                            stop=(c == DCH - 1 and j == cfg.NKC - 1))
                        first = False
                tsum = pa.tile([1, 1], FP32, tag="tsum")
                nc.vector.tensor_reduce(tsum[:], tp[:],
                                        axis=mybir.AxisListType.X, op=ALU.add)
                sig = pa.tile([1, 1], FP32, tag="sig")
                nc.scalar.activation(sig[:], tsum[:], AF.Sigmoid,
                                     bias=bt_t[:], scale=1.0 / S)
                temp = pa.tile([1, 1], FP32, tag="temp")
                nc.vector.tensor_scalar_add(temp[:], sig[:], 0.5)
                invt = pa.tile([1, 1], FP32, tag="invt")
                nc.vector.reciprocal(invt[:], temp[:])
                nc.gpsimd.partition_broadcast(invt128[:], invt[:])

                for j, pt in enumerate(proj_rowmajor(xt, Wk, bias_r["bk"],
                                                     KC, wpa)):
                    kn = pa.tile([128, D], FP16, tag="kn", name="kn", bufs=2)
                    normalize_chunk(pa, pt, kn, None)
                    for p in range(HP):
                        tps = ps.tile([128, 128], FP16, tag="p512", bufs=4,
                                      padded_shape=[128, max(KW, RS)],
                                      name="tps_k")
                        nc.tensor.transpose(
                            tps[:], kn[:, p * 128 : (p + 1) * 128], ident[:])
                        blk = pa.tile([128, 128], FP16, tag="kblk", bufs=4,
                                      name="kblk")
                        if (j + p) % 2 == 0:
                            nc.scalar.activation(blk[:], tps[:], AF.Copy)
                        else:
                            nc.vector.tensor_copy(blk[:], tps[:])
                        nc.sync.dma_start(
                            knt_dram[p][:, j * 128 : (j + 1) * 128], blk[:])

                for j, pt in enumerate(proj_rowmajor(xt, Wv, bias_r["bv"],
                                                     KC, wpa)):
                    nc.vector.tensor_copy(
                        V16[:, j, :, 0:DH],
                        pt[:].rearrange("p (h d) -> p h d", h=NH))

            # ======== phase A2: Q, gate (xT query slice only) ========
            with (
                tc.tile_pool(name="poolA2", bufs=1) as pa2,
                tc.tile_pool(name="wpoolA2", bufs=2) as wpa2,
            ):
                xtq = pa2.tile([128, DCH, RS], FP32R, tag="xtq")
                nc.sync.dma_start(xtq[:], xT[:, :, 0:RS])
                for j, pt in enumerate(proj_rowmajor(xtq, Wq, bias_r["bq"],
                                                     RC, wpa2)):
                    qn = pa2.tile([128, D], FP16, tag="qn", name="qn", bufs=2)
                    normalize_chunk(pa2, pt, qn, invt128[:, 0:1])
                    transpose_to_heads(lambda h: QnT[h], qn, j)
                for j, pt in enumerate(proj_rowmajor(xtq, Wg, bias_r["bg"],
                                                     RC, wpa2)):
                    g16 = pa2.tile([128, D], FP16, tag="g16", name="g16",
                                   bufs=2)
                    nc.scalar.activation(g16[:], pt[:], AF.Sigmoid)
                    nc.sync.dma_start(gate_dram[:, j, :], g16[:])

            # ======== phase B: attention groups ========
            with tc.tile_pool(name="poolB", bufs=1) as pb:
                n_groups = NH // cfg.GROUP
                for g in range(n_groups):
                    heads = list(range(g * cfg.GROUP, (g + 1) * cfg.GROUP))
                    ntile = cfg.GROUP * RC
                    knt = {}
                    for h in heads:
                        t = pb.tile([65, S], FP16, tag=f"knt{h % cfg.GROUP}",
                                    name=f"knt{h}", bufs=2)
                        nc.sync.dma_start(
                            t[0:64, :],
                            knt_dram[h // 2][(h % 2) * 64 : (h % 2) * 64 + 64, :])
                        nc.vector.memset(t[64:65, :], 1.0)
                        knt[h] = t
                    st_t = pb.tile([128, ntile], FP32, tag="st_t", bufs=2)
                    st_lo = pb.tile([128, ntile], FP32, tag="st_lo", bufs=2)
                    st_hi = pb.tile([128, ntile], FP32, tag="st_hi", bufs=2)
                    st_clo = pb.tile([128, ntile], FP32, tag="st_clo", bufs=2)
                    st_chi = pb.tile([128, ntile], FP32, tag="st_chi", bufs=2)
                    st_c = pb.tile([128, ntile], FP32, tag="st_c", bufs=2)
                    nc.vector.memset(st_t[:], 0.1)
                    nc.vector.memset(st_lo[:], -2.1)
                    nc.vector.memset(st_hi[:], 2.1)
                    nc.vector.memset(st_clo[:], float(S))
                    nc.vector.memset(st_chi[:], 0.0)

                    s16 = {}
                    for hi_, h in enumerate(heads):
                        for j in range(RC):
                            srow = pb.tile([128, S], FP16,
                                           tag=f"s16_{hi_}_{j}",
                                           name=f"s16_{hi_}_{j}_g", bufs=1)
                            for kc5 in range(cfg.NKC):
                                pt = ps.tile([128, KW], FP32, tag="p512",
                                             bufs=4,
                                             padded_shape=[128, max(KW, RS)],
                                             name="pt_sr")
                                nc.tensor.matmul(
                                    pt[:],
                                    QnT[h][0:64, j * 128 : (j + 1) * 128],
                                    knt[h][0:64, kc5 * KW : (kc5 + 1) * KW],
                                    start=True, stop=True)
                                if kc5 % 2 == 0:
                                    nc.scalar.activation(
                                        srow[:, kc5 * KW : (kc5 + 1) * KW],
                                        pt[:], AF.Copy)
                                else:
                                    nc.vector.tensor_copy(
                                        srow[:, kc5 * KW : (kc5 + 1) * KW],
                                        pt[:])
                            s16[(hi_, j)] = srow

                    for it in range(cfg.n_sel_iters):
                        for hi_, h in enumerate(heads):
                            for j in range(RC):
                                col = hi_ * RC + j
                                scr = pb.tile([128, S], FP16, tag="selscr",
                                              bufs=2, name="selscr")
                                nc.vector.tensor_scalar(
                                    out=scr[:], in0=s16[(hi_, j)][:],
                                    scalar1=st_t[:, col : col + 1],
                                    scalar2=None,
                                    op0=ALU.is_ge, op1=ALU.add,
                                    accum_out=st_c[:, col : col + 1])
                        islo = pb.tile([128, ntile], U8, tag="islo", bufs=2)
                        nc.vector.tensor_scalar(
                            out=islo[:], in0=st_c[:], scalar1=float(cfg.KK),
                            scalar2=None, op0=ALU.is_ge)
                        nc.vector.copy_predicated(st_lo[:], islo[:], st_t[:])
                        nc.vector.copy_predicated(st_clo[:], islo[:], st_c[:])
                        ishi = pb.tile([128, ntile], U8, tag="ishi", bufs=2)
                        nc.vector.tensor_scalar(
                            out=ishi[:], in0=st_c[:], scalar1=float(cfg.KK),
                            scalar2=None, op0=ALU.is_lt)
                        nc.vector.copy_predicated(st_hi[:], ishi[:], st_t[:])
                        nc.vector.copy_predicated(st_chi[:], ishi[:], st_c[:])
                        tnew = pb.tile([128, ntile], FP32, tag="tnew", bufs=2)
                        if it == 0:
                            nc.vector.tensor_scalar(
                                out=tnew[:], in0=st_c[:],
                                scalar1=float(cfg.KK),
                                scalar2=1.0 / cfg.slope0, op0=ALU.subtract,
                                op1=ALU.mult)
                            nc.vector.tensor_add(tnew[:], tnew[:], st_t[:])
                        else:
                            den = pb.tile([128, ntile], FP32, tag="den",
                                          bufs=2)
                            nc.vector.tensor_sub(den[:], st_clo[:],
                                                 st_chi[:])
                            nc.vector.tensor_scalar_max(den[:], den[:], 1.0)
                            rden = pb.tile([128, ntile], FP32, tag="rden",
                                           bufs=2)
                            nc.vector.reciprocal(rden[:], den[:])
                            nc.vector.tensor_scalar(
                                out=tnew[:], in0=st_clo[:],
                                scalar1=float(cfg.KK),
                                scalar2=None, op0=ALU.subtract)
                            nc.vector.tensor_mul(tnew[:], tnew[:], rden[:])
                            wid = pb.tile([128, ntile], FP32, tag="wid",
                                          bufs=2)
                            nc.vector.tensor_sub(wid[:], st_hi[:], st_lo[:])
                            nc.vector.tensor_mul(tnew[:], tnew[:], wid[:])
                            nc.vector.tensor_add(tnew[:], tnew[:], st_lo[:])
                        nc.vector.tensor_tensor(tnew[:], tnew[:], st_lo[:],
                                                ALU.max)
                        nc.vector.tensor_tensor(tnew[:], tnew[:], st_hi[:],
                                                ALU.min)
                        iseq = pb.tile([128, ntile], U8, tag="iseq", bufs=2)
                        nc.vector.tensor_scalar(
                            out=iseq[:], in0=st_c[:], scalar1=float(cfg.KK),
                            scalar2=None, op0=ALU.not_equal)
                        nc.vector.copy_predicated(st_t[:], iseq[:], tnew[:])

                    tneg = pb.tile([128, ntile], FP32, tag="tneg", bufs=2)
                    nc.vector.tensor_scalar(
                        out=tneg[:], in0=st_t[:], scalar1=invt128[:, 0:1],
                        scalar2=-1.0, op0=ALU.mult, op1=ALU.mult)
                    for hi_, h in enumerate(heads):
                        for j in range(RC):
                            col = hi_ * RC + j
                            nc.gpsimd.dma_start(
                                out=QnT[h][64:65, j * 128 : (j + 1) * 128],
                                in_=tneg[:, col : col + 1])

                    for hi_, h in enumerate(heads):
                        avp = ps.tile([65, RS], FP32, tag="p512", bufs=4,
                                      padded_shape=[128, max(KW, RS)],
                                      name="avp")
                        for kc in range(KC):
                            stp = ps.tile([128, RS], FP32, tag="p512",
                                          bufs=4,
                                          padded_shape=[128, max(KW, RS)],
                                          name="stp")
                            nc.tensor.matmul(
                                stp[:], knt[h][:, kc * 128 : (kc + 1) * 128],
                                QnT[h][:], start=True, stop=True)
                            e16 = pb.tile([128, RS], FP16, tag="e16",
                                          bufs=3, name="e16")
                            nc.scalar.activation(e16[:], stp[:], AF.Exp)
                            em16 = pb.tile([128, RS], FP16, tag="em16",
                                           bufs=3, name="em16")
                            nc.vector.scalar_tensor_tensor(
                                out=em16[:], in0=e16[:], scalar=1.0,
                                in1=e16[:], op0=ALU.is_ge, op1=ALU.mult)
                            nc.tensor.matmul(
                                avp[:], V16[:, kc, h, :], em16[:],
                                start=(kc == 0), stop=(kc == KC - 1))
                        zrow = pb.tile([1, RS], FP32, tag="zrow", bufs=2)
                        nc.vector.tensor_scalar_max(zrow[:], avp[64:65, :],
                                                    0.5)
                        zrec = pb.tile([1, RS], FP32, tag="zrec", bufs=2)
                        nc.vector.reciprocal(zrec[:], zrow[:])
                        zrep = pb.tile([64, RS], FP32, tag="zrep", bufs=2)
                        nc.gpsimd.partition_broadcast(zrep[:], zrec[:])
                        nc.vector.tensor_tensor(
                            attnT[(h % 2) * 64 : (h % 2) * 64 + 64,
                                  h // 2, :],
                            avp[0:64, :], zrep[:], ALU.mult)

            # ======== phase C: output projection + gate ========
            with tc.tile_pool(name="poolC", bufs=1) as pc:
                wo_t = pc.tile([128, HP, D], FP16, tag="wo")
                nc.sync.dma_start(wo_t[:], Wo[:])
                xs_t = pc.tile([128, RC, D], FP32, tag="xs")
                nc.sync.dma_start(xs_t[:],
                                  xs.rearrange("(c p) d -> p c d", p=128))
                gr = pc.tile([128, RC, D], FP16, tag="gr")
                nc.sync.dma_start(gr[:], gate_dram[:])
                for j in range(RC):
                    op = ps.tile([128, D], FP32, tag="projp", bufs=2,
                                 name="op_out")
                    for n in range(D // NW):
                        for p in range(HP):
                            nc.tensor.matmul(
                                op[:, n * NW : (n + 1) * NW],
                                attnT[:, p, j * 128 : (j + 1) * 128],
                                wo_t[:, p, n * NW : (n + 1) * NW],
                                start=(p == 0), stop=False)
                        nc.tensor.matmul(
                            op[:, n * NW : (n + 1) * NW], ones_h[:],
                            bo_t[:, n * NW : (n + 1) * NW], start=False,
                            stop=True)
                    dd = pc.tile([128, D], FP32, tag="dd", bufs=2, name="dd")
                    nc.vector.tensor_sub(dd[:], op[:], xs_t[:, j, :])
                    nc.vector.tensor_mul(dd[:], dd[:], gr[:, j, :])
                    oo = pc.tile([128, D], FP32, tag="oo", bufs=2, name="oo")
                    nc.vector.tensor_add(oo[:], dd[:], xs_t[:, j, :])
                    nc.sync.dma_start(
                        out.rearrange("(c p) d -> p c d", p=128)[:, j, :],
                        oo[:])

    nc.finalize()
    return nc


# ---------------------------------------------------------------------------
_NC_CACHE = {}
LAST_EXEC_NS = None
LAST_RESULTS = None


def _get_nc(cfg_key=None):
    if cfg_key not in _NC_CACHE:
        _NC_CACHE[cfg_key] = build(Cfg())
    return _NC_CACHE[cfg_key]


def _pack_core_inputs(x, Wq, bq, Wk, bk, Wv, bv, Wo, bo, Wt, bt, Wg, bg,
                      b, r0, cfg):
    S, D, RS, DCH, HP = cfg.S, cfg.D, cfg.RS, cfg.DCH, cfg.HP
    xb = x[b]
    xt = np.ascontiguousarray(
        np.roll(xb.T, -r0, axis=1).reshape(DCH, 128, S).transpose(1, 0, 2))
    xss = np.ascontiguousarray(xb[r0 : r0 + RS])
    def wpack(W):
        return np.ascontiguousarray(W.reshape(DCH, 128, D).transpose(1, 0, 2))
    return {
        "xT": xt.astype(np.float32),
        "xs": xss.astype(np.float32),
        "Wq": wpack(Wq).astype(np.float32),
        "Wk": wpack(Wk).astype(np.float32),
        "Wv": wpack(Wv).astype(np.float32),
        "Wg": wpack(Wg).astype(np.float32),
        "Wo": np.ascontiguousarray(
            Wo.reshape(HP, 128, D).transpose(1, 0, 2)).astype(np.float16),
        "Wt": np.ascontiguousarray(Wt.reshape(DCH, 128).T).astype(np.float32),
        "bq": bq.reshape(1, D).astype(np.float32),
        "bk": bk.reshape(1, D).astype(np.float32),
        "bv": bv.reshape(1, D).astype(np.float32),
        "bg": bg.reshape(1, D).astype(np.float32),
        "bo": bo.reshape(1, D).astype(np.float16),
        "bt": bt.reshape(1, 1).astype(np.float32),
    }


def kernel(**inputs):
    from concourse.bass_utils import run_bass_kernel_spmd
    cfg = Cfg()
    x = np.asarray(inputs["x"], np.float32)
    B, S, D = x.shape
    nc = _get_nc()
    in_maps = []
    for c in range(8):
        b, q = c // 4, c % 4
        in_maps.append(_pack_core_inputs(
            x, np.asarray(inputs["Wq"]), np.asarray(inputs["bq"]),
            np.asarray(inputs["Wk"]), np.asarray(inputs["bk"]),
            np.asarray(inputs["Wv"]), np.asarray(inputs["bv"]),
            np.asarray(inputs["Wo"]), np.asarray(inputs["bo"]),
            np.asarray(inputs["Wt"]), np.asarray(inputs["bt"]),
            np.asarray(inputs["Wg"]), np.asarray(inputs["bg"]),
            b, q * cfg.RS, cfg))
    trace = bool(int(os.environ.get("KERNEL_TRACE", "0")))
    res = run_bass_kernel_spmd(nc, in_maps, core_ids=list(range(8)),
                               trace=trace)
    global LAST_EXEC_NS, LAST_RESULTS
    LAST_EXEC_NS = res.exec_time_ns
    LAST_RESULTS = res
    out = np.empty((B, S, D), np.float32)
    for c in range(8):
        b, q = c // 4, c % 4
        out[b, q * cfg.RS : (q + 1) * cfg.RS] = res.results[c]["out"]
    return out



# revision 24
# speedup vs baseline: 1.8521x; 1.8521x over previous
"""EvolvedAttention Trainium2 Bass kernel (v2).

Full inputs -> full output. Sharding: 8 cores = 2 batches x 4 query-row
slices. Each core computes K/V/attention for its (batch, row-slice) with
all 16 heads; host slices inputs and concatenates row-slice outputs.

v2 design (from ntff trace of v1: DVE 73% busy on top-k counting, PE 38%
and cold):
  - Q/K/V projections in fp8e4 + DoubleRow (weights x32 host-side, folded
    back via Wo/32; cosine normalization cancels the scale for q/k).
  - gate/temp/Wo in fp16.
  - KnT (head-major [65, S], ones row for the threshold trick) and the
    gate stay SBUF-resident; no DRAM staging.
  - top-k threshold found on 4x-subsampled keys (strided matmul rhs),
    3 count-iterations split across ACT (Sign+accum), GPSIMD and DVE,
    bracketed false-position in "acc" space (acc = #ge - #lt).
  - scores recomputed transposed with threshold folded in (K=65), exp on
    ACT PSUM->fp8, mask on GPSIMD (em8 = [z>=0]*e8), AV in fp8 DoubleRow
    with a ones column in V8 for the softmax denominator.
  - selection of group g pipelines against attention of group g-1; the
    V projection fills the group-0 selection bubble.
"""

import os
import numpy as np

import concourse.bass as bass
import concourse.mybir as mybir
import concourse.tile as tile
from concourse import bacc

FP32 = mybir.dt.float32
FP16 = mybir.dt.float16
FP8 = mybir.dt.float8e4
U8 = mybir.dt.uint8
AF = mybir.ActivationFunctionType
ALU = mybir.AluOpType
DR = mybir.MatmulPerfMode.DoubleRow

WSCALE = 32.0


class Cfg:
    def __init__(self):
        self.S = 2048
        self.D = 1024
        self.NH = 16
        self.DH = 64
        self.RS = 512
        self.KK = self.S // 4          # top-k
        self.SUB = 4                   # key subsample for threshold search
        self.SS = self.S // self.SUB   # sampled keys (512)
        self.DCH = self.D // 128       # 8
        self.KC = self.S // 128        # 16
        self.RC = self.RS // 128       # 4
        self.HP = self.NH // 2         # 8
        self.GROUP = 4
        self.NG = self.NH // self.GROUP
        self.n_sel_iters = 3
        # target in acc space: acc = 2*c - SS, c target = KK/SUB
        self.ATGT = float(2 * (self.KK // self.SUB) - self.SS)  # -256
        self.slope0 = 2.0 * 2.8 * self.SS  # d(acc)/dt estimate


def build(cfg: Cfg, with_bias: bool):
    nc = bacc.Bacc()
    S, D, NH, DH, RS = cfg.S, cfg.D, cfg.NH, cfg.DH, cfg.RS
    DCH, KC, RC, HP = cfg.DCH, cfg.KC, cfg.RC, cfg.HP
    SS, G, NG = cfg.SS, cfg.GROUP, cfg.NG

    x8T = nc.dram_tensor("x8T", [128, DCH, S], FP8, kind="ExternalInput")
    x16T = nc.dram_tensor("x16T", [128, DCH, RS], FP16, kind="ExternalInput")
    xs = nc.dram_tensor("xs", [128, RC, D], FP32, kind="ExternalInput")
    Wq = nc.dram_tensor("Wq", [128, DCH, D], FP8, kind="ExternalInput")
    Wk = nc.dram_tensor("Wk", [128, DCH, D], FP8, kind="ExternalInput")
    Wv = nc.dram_tensor("Wv", [128, DCH, D], FP8, kind="ExternalInput")
    Wg = nc.dram_tensor("Wg", [128, DCH, D], FP16, kind="ExternalInput")
    Wo = nc.dram_tensor("Wo", [128, HP, D], FP16, kind="ExternalInput")
    Wt = nc.dram_tensor("Wt", [128, DCH], FP8, kind="ExternalInput")
    bt = nc.dram_tensor("bt", [1, 1], FP32, kind="ExternalInput")
    if with_bias:
        bq = nc.dram_tensor("bq", [1, D], FP16, kind="ExternalInput")
        bk = nc.dram_tensor("bk", [1, D], FP16, kind="ExternalInput")
        bv = nc.dram_tensor("bv", [1, D], FP16, kind="ExternalInput")
        bg = nc.dram_tensor("bg", [1, D], FP16, kind="ExternalInput")
        bo = nc.dram_tensor("bo", [1, D], FP16, kind="ExternalInput")
    out = nc.dram_tensor("out", [128, RC, D], FP32, kind="ExternalOutput")

    with tile.TileContext(nc) as tc:
        with (
            tc.tile_pool(name="persist", bufs=1) as pp,
            tc.tile_pool(name="psum", bufs=2, space="PSUM") as ps,
        ):
            # ---------------- persistent tiles (phase A) ----------------
            ident = pp.tile([128, 128], FP16, tag="ident")
            from concourse.masks import make_identity
            make_identity(nc, ident[:])
            ones_h = pp.tile([1, 128], FP16, tag="ones_h")
            nc.vector.memset(ones_h[:], 1.0)
            KnT = [pp.tile([65, S], FP16, tag=f"knt{h}", name=f"knt{h}")
                   for h in range(NH)]
            QnT = [pp.tile([65, RS], FP16, tag=f"qnt{h}", name=f"qnt{h}")
                   for h in range(NH)]
            gate16 = pp.tile([128, RC, D], FP16, tag="gate16")
            invt128 = pp.tile([128, 1], FP32, tag="invt128")
            bt_t = pp.tile([1, 1], FP32, tag="bt")
            nc.sync.dma_start(bt_t[:], bt[:])
            wt_t = pp.tile([128, DCH], FP8, tag="wt")
            nc.sync.dma_start(wt_t[:], Wt[:])
            bias_t = {}
            if with_bias:
                for nm, dram in (("bq", bq), ("bk", bk), ("bv", bv),
                                 ("bg", bg), ("bo", bo)):
                    t = pp.tile([1, D], FP16, tag=nm, name=f"b_{nm}")
                    nc.sync.dma_start(t[:], dram[:])
                    bias_t[nm] = t

            def psum512(name):
                return ps.tile([128, 512], FP32, tag="p512", bufs=4,
                               padded_shape=[128, 512], name=name)

            # ---------------- helpers ----------------
            def proj_fp8(xt8, w_dram, bias_row, n_chunks, wpool, wtag):
                """fp8 DoubleRow projection; yields (j, ptA, ptB) psum pairs
                ([128,512] each, together the [128,1024] row-chunk)."""
                w = wpool.tile([128, DCH, D], FP8, tag=wtag, name=wtag,
                               bufs=2)
                nc.sync.dma_start(w[:], w_dram[:])
                for j in range(n_chunks):
                    pts = []
                    for n in range(2):
                        pt = psum512(f"pt_{wtag}{n}")
                        for cp in range(DCH // 2):
                            nc.tensor.matmul(
                                pt[:],
                                xt8[:, 2 * cp : 2 * cp + 2,
                                    j * 128 : (j + 1) * 128],
                                w[:, 2 * cp : 2 * cp + 2,
                                  n * 512 : (n + 1) * 512],
                                start=(cp == 0),
                                stop=(cp == DCH // 2 - 1 and bias_row is None),
                                perf_mode=DR)
                        if bias_row is not None:
                            nc.tensor.matmul(
                                pt[:], ones_h[:],
                                bias_row[:, n * 512 : (n + 1) * 512],
                                start=False, stop=True)
                        pts.append(pt)
                    yield j, pts[0], pts[1]

            def proj_fp16_half(xt16, w_dram, bias_row, n, n_chunks, wpool,
                               wtag):
                """one 512-wide output half of an fp16 projection; yields
                (j, pt) psum tiles [128,512]."""
                w = wpool.tile([128, DCH, 512], FP16, tag=wtag, name=wtag,
                               bufs=2)
                nc.sync.dma_start(w[:], w_dram[:, :, n * 512 : (n + 1) * 512])
                for j in range(n_chunks):
                    pt = psum512(f"pt_{wtag}{n}")
                    for c in range(DCH):
                        nc.tensor.matmul(
                            pt[:],
                            xt16[:, c, j * 128 : (j + 1) * 128],
                            w[:, c, :],
                            start=(c == 0),
                            stop=(c == DCH - 1 and bias_row is None))
                    if bias_row is not None:
                        nc.tensor.matmul(
                            pt[:], ones_h[:],
                            bias_row[:, n * 512 : (n + 1) * 512],
                            start=False, stop=True)
                    yield j, pt

            def normalize_pair(sp, ptA, ptB, dst16, extra_scale_ap):
                """cosine-normalize a [128,1024] row-chunk given as two
                [128,512] psum halves; dst16 [128, D] fp16."""
                sq = sp.tile([128, D], FP16, tag="sq", name="sq", bufs=2)
                nc.scalar.activation(sq[:, 0:512], ptA[:], AF.Square)
                nc.scalar.activation(sq[:, 512:1024], ptB[:], AF.Square)
                n2 = sp.tile([128, NH], FP32, tag="n2", name="n2", bufs=2)
                nc.vector.tensor_reduce(
                    n2[:], sq[:].rearrange("p (h d) -> p h d", h=NH),
                    axis=mybir.AxisListType.X, op=ALU.add)
                nc.vector.tensor_scalar_max(n2[:], n2[:], 1e-12)
                rec = sp.tile([128, NH], FP32, tag="rec", name="rec", bufs=2)
                nc.vector.reciprocal(rec[:], n2[:])
                rsq = sp.tile([128, NH], FP32, tag="rsq", name="rsq", bufs=2)
                nc.scalar.activation(rsq[:], rec[:], AF.Sqrt)
                if extra_scale_ap is not None:
                    nc.vector.tensor_scalar(
                        out=rsq[:], in0=rsq[:], scalar1=extra_scale_ap,
                        scalar2=None, op0=ALU.mult)
                for half, pt in ((0, ptA), (1, ptB)):
                    nc.vector.tensor_tensor(
                        dst16[:, half * 512 : (half + 1) * 512]
                            .rearrange("p (h d) -> p h d", h=NH // 2),
                        pt[:].rearrange("p (h d) -> p h d", h=NH // 2),
                        rsq[:, half * 8 : half * 8 + 8]
                            .rearrange("p (h o) -> p h o", o=1)
                            .to_broadcast([128, NH // 2, DH]),
                        ALU.mult)

            def transpose_to_heads(dst_of_head, src16, j, who):
                """src16 [128 rows, 1024] -> per-head [64, 128] blocks into
                dst_of_head(h)[0:64, j*128:(j+1)*128]."""
                for p in range(HP):
                    tps = ps.tile([128, 128], FP16, tag="p512", bufs=4,
                                  padded_shape=[128, 512], name=f"tps_{who}")
                    nc.tensor.transpose(
                        tps[:], src16[:, p * 128 : (p + 1) * 128], ident[:])
                    for hh in range(2):
                        h = 2 * p + hh
                        dst = dst_of_head(h)[0:64, j * 128 : (j + 1) * 128]
                        src = tps[hh * 64 : hh * 64 + 64, :]
                        if (p + hh + j) % 2 == 0:
                            nc.scalar.activation(dst, src, AF.Copy)
                        else:
                            nc.vector.tensor_copy(dst, src)

            # ================ phase A ================
            with tc.tile_pool(name="poolX8", bufs=1) as px:
                xt8 = px.tile([128, DCH, S], FP8, tag="xt8")
                nc.sync.dma_start(xt8[:], x8T[:])

                with (
                    tc.tile_pool(name="poolA", bufs=1) as pa,
                    tc.tile_pool(name="wpoolA", bufs=2) as wpa,
                ):
                    xt16 = pa.tile([128, DCH, RS], FP16, tag="xt16")
                    nc.sync.dma_start(xt16[:], x16T[:])

                    # --- K projection -> KnT (resident) ---
                    for j, ptA, ptB in proj_fp8(xt8, Wk, bias_t.get("bk"),
                                                KC, wpa, "w8"):
                        kn = pa.tile([128, D], FP16, tag="kn", name="kn",
                                     bufs=2)
                        normalize_pair(pa, ptA, ptB, kn, None)
                        transpose_to_heads(lambda h: KnT[h], kn, j, "k")

                    # --- temp (from fp8 x; scale folded into sigmoid) ---
                    tp = ps.tile([1, 512], FP32, tag="p512", bufs=4,
                                 padded_shape=[128, 512], name="tp_temp")
                    first = True
                    for c in range(DCH):
                        for j in range(4):
                            nc.tensor.matmul(
                                tp[:], wt_t[:, c : c + 1],
                                xt8[:, c, j * 512 : (j + 1) * 512],
                                start=first,
                                stop=(c == DCH - 1 and j == 3))
                            first = False
                    tsum = pa.tile([1, 1], FP32, tag="tsum")
                    nc.vector.tensor_reduce(tsum[:], tp[:],
                                            axis=mybir.AxisListType.X,
                                            op=ALU.add)
                    sig = pa.tile([1, 1], FP32, tag="sig")
                    nc.scalar.activation(sig[:], tsum[:], AF.Sigmoid,
                                         bias=bt_t[:],
                                         scale=1.0 / (S * WSCALE))
                    temp = pa.tile([1, 1], FP32, tag="temp")
                    nc.vector.tensor_scalar_add(temp[:], sig[:], 0.5)
                    invt = pa.tile([1, 1], FP32, tag="invt")
                    nc.vector.reciprocal(invt[:], temp[:])
                    nc.gpsimd.partition_broadcast(invt128[:], invt[:])

                    # --- Q projection -> QnT (1/temp folded in) ---
                    for j, ptA, ptB in proj_fp8(xt8, Wq, bias_t.get("bq"),
                                                RC, wpa, "w8"):
                        qn = pa.tile([128, D], FP16, tag="qn", name="qn",
                                     bufs=2)
                        normalize_pair(pa, ptA, ptB, qn, invt128[:, 0:1])
                        transpose_to_heads(lambda h: QnT[h], qn, j, "q")

                    # --- gate (fp16, query slice only, resident) ---
                    for n in range(2):
                        for j, pt in proj_fp16_half(
                                xt16, Wg, bias_t.get("bg"), n, RC, wpa,
                                "wg16h"):
                            nc.scalar.activation(
                                gate16[:, j, n * 512 : (n + 1) * 512],
                                pt[:], AF.Sigmoid)

                # ---- late persistent tiles (group phase) ----
                V8 = pp.tile([128, KC, NH, 66], FP8, tag="v8")
                nc.vector.memset(V8[:, :, :, 64:66], 1.0)
                attnT = pp.tile([128, HP, RS], FP16, tag="attnT")
                for h in range(NH):
                    nc.gpsimd.memset(KnT[h][64:65, :], 1.0)

                # ============ selection / attention bodies ============
                def selection(gi, gp):
                    heads = list(range(gi * G, (gi + 1) * G))
                    nt = G * RC
                    nt0 = gp.tile([128, 1], FP32, tag="nt0")
                    nc.vector.memset(nt0[:], -0.1)
                    st_t = gp.tile([128, nt], FP32, tag="st_t")
                    st_lo = gp.tile([128, nt], FP32, tag="st_lo")
                    st_hi = gp.tile([128, nt], FP32, tag="st_hi")
                    st_clo = gp.tile([128, nt], FP32, tag="st_clo")
                    st_chi = gp.tile([128, nt], FP32, tag="st_chi")
                    acc = gp.tile([128, nt], FP32, tag="acc")
                    nc.vector.memset(st_t[:], 0.1)
                    nc.vector.memset(st_lo[:], -2.1)
                    nc.vector.memset(st_hi[:], 2.1)
                    nc.vector.memset(st_clo[:], float(SS))
                    nc.vector.memset(st_chi[:], float(-SS))

                    # selection scores (subsampled keys, strided rhs)
                    s16 = {}
                    for hi_, h in enumerate(heads):
                        for j in range(RC):
                            sp_ = psum512(f"selp_{hi_}_{j}")
                            nc.tensor.matmul(
                                sp_[:],
                                QnT[h][0:64, j * 128 : (j + 1) * 128],
                                KnT[h][0:64, 1 : S : cfg.SUB],
                                start=True, stop=True)
                            srow = gp.tile([128, SS], FP16,
                                           tag=f"s16_{hi_}_{j}",
                                           name=f"s16_{hi_}_{j}")
                            if (hi_ + j) % 2 == 0:
                                nc.scalar.activation(srow[:], sp_[:],
                                                     AF.Copy)
                            else:
                                nc.vector.tensor_copy(srow[:], sp_[:])
                            s16[(hi_, j)] = srow

                    negt = gp.tile([128, nt], FP32, tag="negt")
                    for it in range(cfg.n_sel_iters):
                        use_act = (it != 1)
                        if it == 2:
                            nc.vector.tensor_scalar(
                                out=negt[:], in0=st_t[:], scalar1=-1.0,
                                scalar2=None, op0=ALU.mult)
                        for hi_, h in enumerate(heads):
                            for j in range(RC):
                                col = hi_ * RC + j
                                if use_act:
                                    bias_ap = (nt0[:, 0:1] if it == 0
                                               else negt[:, col : col + 1])
                                    scr = gp.tile([128, SS], FP8, tag="scr8",
                                                  bufs=2, name="scr8")
                                    nc.scalar.activation(
                                        scr[:], s16[(hi_, j)][:], AF.Sign,
                                        bias=bias_ap,
                                        accum_out=acc[:, col : col + 1])
                                else:
                                    scr = gp.tile([128, SS], FP16,
                                                  tag="scr16", bufs=2,
                                                  name="scr16")
                                    nc.vector.tensor_scalar(
                                        out=scr[:], in0=s16[(hi_, j)][:],
                                        scalar1=st_t[:, col : col + 1],
                                        scalar2=None, op0=ALU.is_ge,
                                        op1=ALU.add,
                                        accum_out=acc[:, col : col + 1])
                        if not use_act:
                            # c in [0,SS] -> acc = 2c - SS
                            nc.vector.tensor_scalar(
                                out=acc[:], in0=acc[:], scalar1=2.0,
                                scalar2=float(-SS), op0=ALU.mult,
                                op1=ALU.add)
                        islo = gp.tile([128, nt], U8, tag="islo", bufs=2)
                        nc.vector.tensor_scalar(
                            out=islo[:], in0=acc[:], scalar1=cfg.ATGT,
                            scalar2=None, op0=ALU.is_ge)
                        nc.vector.copy_predicated(st_lo[:], islo[:], st_t[:])
                        nc.vector.copy_predicated(st_clo[:], islo[:], acc[:])
                        ishi = gp.tile([128, nt], U8, tag="ishi", bufs=2)
                        nc.vector.tensor_scalar(
                            out=ishi[:], in0=acc[:], scalar1=cfg.ATGT,
                            scalar2=None, op0=ALU.is_lt)
                        nc.vector.copy_predicated(st_hi[:], ishi[:], st_t[:])
                        nc.vector.copy_predicated(st_chi[:], ishi[:], acc[:])
                        tnew = gp.tile([128, nt], FP32, tag="tnew", bufs=2)
                        if it == 0:
                            nc.vector.tensor_scalar(
                                out=tnew[:], in0=acc[:], scalar1=cfg.ATGT,
                                scalar2=1.0 / cfg.slope0, op0=ALU.subtract,
                                op1=ALU.mult)
                            nc.vector.tensor_add(tnew[:], tnew[:], st_t[:])
                        else:
                            den = gp.tile([128, nt], FP32, tag="den",
                                          bufs=2)
                            nc.vector.tensor_sub(den[:], st_clo[:],
                                                 st_chi[:])
                            nc.vector.tensor_scalar_max(den[:], den[:], 1.0)
                            rden = gp.tile([128, nt], FP32, tag="rden",
                                           bufs=2)
                            nc.vector.reciprocal(rden[:], den[:])
                            nc.vector.tensor_scalar(
                                out=tnew[:], in0=st_clo[:],
                                scalar1=cfg.ATGT,
                                scalar2=None, op0=ALU.subtract)
                            nc.vector.tensor_mul(tnew[:], tnew[:], rden[:])
                            wid = gp.tile([128, nt], FP32, tag="wid",
                                          bufs=2)
                            nc.vector.tensor_sub(wid[:], st_hi[:], st_lo[:])
                            nc.vector.tensor_mul(tnew[:], tnew[:], wid[:])
                            nc.vector.tensor_add(tnew[:], tnew[:], st_lo[:])
                        nc.vector.tensor_tensor(tnew[:], tnew[:], st_lo[:],
                                                ALU.max)
                        nc.vector.tensor_tensor(tnew[:], tnew[:], st_hi[:],
                                                ALU.min)
                        iseq = gp.tile([128, nt], U8, tag="iseq", bufs=2)
                        nc.vector.tensor_scalar(
                            out=iseq[:], in0=acc[:], scalar1=cfg.ATGT,
                            scalar2=None, op0=ALU.not_equal)
                        nc.vector.copy_predicated(st_t[:], iseq[:], tnew[:])

                    # tneg = -t (scores already in /temp units); write into
                    # QnT[h] row 64 via PE transpose + small copies
                    tneg = gp.tile([128, nt], FP16, tag="tneg")
                    nc.vector.tensor_scalar(
                        out=tneg[:], in0=st_t[:], scalar1=-1.0,
                        scalar2=None, op0=ALU.mult)
                    ttp = ps.tile([nt, 128], FP16, tag="p512", bufs=4,
                                  padded_shape=[128, 512], name="ttp")
                    nc.tensor.transpose(ttp[:], tneg[:], ident[:])
                    tnT = gp.tile([nt, 128], FP16, tag="tnT")
                    nc.scalar.activation(tnT[:], ttp[:], AF.Copy)
                    for hi_, h in enumerate(heads):
                        for j in range(RC):
                            col = hi_ * RC + j
                            nc.sync.dma_start(
                                QnT[h][64:65, j * 128 : (j + 1) * 128],
                                tnT[col : col + 1, :])

                def attention(gi):
                    heads = list(range(gi * G, (gi + 1) * G))
                    for hi_, h in enumerate(heads):
                        avp = ps.tile([65, RS], FP32, tag="avp", bufs=2,
                                      padded_shape=[128, 512], name="avp")
                        for kcp in range(KC // 2):
                            em8 = pp.tile([128, 2, RS], FP8, tag="em8",
                                          bufs=4, name="em8")
                            for sub in range(2):
                                kc = 2 * kcp + sub
                                stp = psum512("stp")
                                nc.tensor.matmul(
                                    stp[:],
                                    KnT[h][:, kc * 128 : (kc + 1) * 128],
                                    QnT[h][:], start=True, stop=True)
                                e16 = pp.tile([128, RS], FP16, tag="e16",
                                              bufs=4, name="e16")
                                nc.scalar.activation(e16[:], stp[:], AF.Exp)
                                nc.vector.scalar_tensor_tensor(
                                    out=em8[:, sub, :], in0=e16[:],
                                    scalar=1.0, in1=e16[:],
                                    op0=ALU.is_ge, op1=ALU.mult)
                            nc.tensor.matmul(
                                avp[:],
                                V8[:, 2 * kcp : 2 * kcp + 2, h, 0:65],
                                em8[:, :, :],
                                start=(kcp == 0), stop=(kcp == KC // 2 - 1),
                                perf_mode=DR)
                        # normalize: attnT = avp[0:64] * (1/z)
                        zrow = pp.tile([1, RS], FP32, tag="zrow", bufs=2,
                                       name="zrow")
                        nc.vector.tensor_scalar_max(zrow[:], avp[64:65, :],
                                                    1e-30)
                        zrec = pp.tile([1, RS], FP32, tag="zrec", bufs=2,
                                       name="zrec")
                        nc.vector.reciprocal(zrec[:], zrow[:])
                        zrep = pp.tile([64, RS], FP32, tag="zrep", bufs=2,
                                       name="zrep")
                        nc.gpsimd.partition_broadcast(zrep[:], zrec[:])
                        nc.vector.tensor_tensor(
                            attnT[(h % 2) * 64 : (h % 2) * 64 + 64,
                                  h // 2, :],
                            avp[0:64, :], zrep[:], ALU.mult)

                # ============ pipeline: selection(g) | attention(g-1) ====
                with tc.tile_pool(name="poolG0", bufs=1) as gp0:
                    selection(0, gp0)
                with tc.tile_pool(name="poolV", bufs=1) as pv:
                    for j, ptA, ptB in proj_fp8(xt8, Wv, bias_t.get("bv"),
                                                KC, pv, "wv8"):
                        for half, pt in ((0, ptA), (1, ptB)):
                            dst = V8[:, j, half * 8 : half * 8 + 8, 0:64]
                            src = pt[:].rearrange("p (h d) -> p h d", h=8)
                            if (j + half) % 2 == 0:
                                nc.scalar.activation(dst, src, AF.Copy)
                            else:
                                nc.vector.tensor_copy(dst, src)

            # poolX8 closed (xt8 freed)
            for gi in range(1, NG):
                with tc.tile_pool(name=f"poolG{gi}", bufs=1) as gp_:
                    selection(gi, gp_)
                attention(gi - 1)
            attention(NG - 1)

            # ================ phase C: out proj + gate ================
            with tc.tile_pool(name="poolC", bufs=1) as pc:
                wo_t = pc.tile([128, HP, D], FP16, tag="wo")
                nc.sync.dma_start(wo_t[:], Wo[:])
                xs_t = pc.tile([128, RC, D], FP32, tag="xs")
                nc.sync.dma_start(xs_t[:], xs[:])
                for j in range(RC):
                    for n in range(2):
                        op = psum512(f"op_out{n}")
                        for p in range(HP):
                            nc.tensor.matmul(
                                op[:],
                                attnT[:, p, j * 128 : (j + 1) * 128],
                                wo_t[:, p, n * 512 : (n + 1) * 512],
                                start=(p == 0),
                                stop=(p == HP - 1 and not with_bias))
                        if with_bias:
                            nc.tensor.matmul(
                                op[:], ones_h[:],
                                bias_t["bo"][:, n * 512 : (n + 1) * 512],
                                start=False, stop=True)
                        sl = slice(n * 512, (n + 1) * 512)
                        dd = pc.tile([128, 512], FP32, tag="dd", bufs=3,
                                     name="dd")
                        nc.vector.tensor_sub(dd[:], op[:], xs_t[:, j, sl])
                        nc.vector.tensor_mul(dd[:], dd[:], gate16[:, j, sl])
                        oo = pc.tile([128, 512], FP32, tag="oo", bufs=3,
                                     name="oo")
                        nc.gpsimd.tensor_add(oo[:], dd[:], xs_t[:, j, sl])
                        nc.sync.dma_start(out[:, j, sl], oo[:])

    nc.finalize()
    return nc


# ---------------------------------------------------------------------------
_NC_CACHE = {}
LAST_EXEC_NS = None
LAST_RESULTS = None


def _get_nc(with_bias: bool):
    key = bool(with_bias)
    if key not in _NC_CACHE:
        _NC_CACHE[key] = build(Cfg(), key)
    return _NC_CACHE[key]


def _pack_core_inputs(x, Wq, bq, Wk, bk, Wv, bv, Wo, bo, Wt, bt, Wg, bg,
                      b, r0, cfg, with_bias, fp8):
    S, D, RS, DCH, HP = cfg.S, cfg.D, cfg.RS, cfg.DCH, cfg.HP
    xb = x[b]
    xt = np.ascontiguousarray(
        np.roll(xb.T, -r0, axis=1).reshape(DCH, 128, S).transpose(1, 0, 2))
    xss = np.ascontiguousarray(
        xb[r0 : r0 + RS].reshape(cfg.RC, 128, D).transpose(1, 0, 2))

    def wpack(W, dt, scale=1.0):
        return np.ascontiguousarray(
            (W * scale).reshape(DCH, 128, D).transpose(1, 0, 2)).astype(dt)

    m = {
        "x8T": xt.astype(fp8),
        "x16T": np.ascontiguousarray(xt[:, :, 0:RS]).astype(np.float16),
        "xs": xss.astype(np.float32),
        "Wq": wpack(Wq, fp8, WSCALE),
        "Wk": wpack(Wk, fp8, WSCALE),
        "Wv": wpack(Wv, fp8, WSCALE),
        "Wg": wpack(Wg, np.float16),
        "Wo": np.ascontiguousarray(
            (Wo / WSCALE).reshape(HP, 128, D).transpose(1, 0, 2))
            .astype(np.float16),
        "Wt": np.ascontiguousarray(
            Wt.reshape(DCH, 128).T * WSCALE).astype(fp8),
        "bt": bt.reshape(1, 1).astype(np.float32),
    }
    if with_bias:
        m["bq"] = (bq * WSCALE).reshape(1, D).astype(np.float16)
        m["bk"] = (bk * WSCALE).reshape(1, D).astype(np.float16)
        m["bv"] = (bv * WSCALE).reshape(1, D).astype(np.float16)
        m["bg"] = bg.reshape(1, D).astype(np.float16)
        m["bo"] = bo.reshape(1, D).astype(np.float16)
    return m


def kernel(**inputs):
    from concourse.bass_utils import run_bass_kernel_spmd
    cfg = Cfg()
    fp8 = mybir.dt.np(FP8)
    x = np.asarray(inputs["x"], np.float32)
    B, S, D = x.shape
    args = [np.asarray(inputs[k]) for k in
            ("Wq", "bq", "Wk", "bk", "Wv", "bv", "Wo", "bo", "Wt", "bt",
             "Wg", "bg")]
    with_bias = any(np.any(np.asarray(inputs[k])) for k in
                    ("bq", "bk", "bv", "bg", "bo"))
    nc = _get_nc(with_bias)
    in_maps = []
    for c in range(8):
        b, q = c // 4, c % 4
        in_maps.append(_pack_core_inputs(
            x, *args, b, q * cfg.RS, cfg, with_bias, fp8))
    trace = bool(int(os.environ.get("KERNEL_TRACE", "0")))
    res = run_bass_kernel_spmd(nc, in_maps, core_ids=list(range(8)),
                               trace=trace)
    global LAST_EXEC_NS, LAST_RESULTS
    LAST_EXEC_NS = res.exec_time_ns
    LAST_RESULTS = res
    out = np.empty((B, S, D), np.float32)
    for c in range(8):
        b, q = c // 4, c % 4
        o = res.results[c]["out"]  # [128, RC, D]
        out[b, q * cfg.RS : (q + 1) * cfg.RS] = \
            o.transpose(1, 0, 2).reshape(cfg.RS, D)
    return out


# revision 40
# speedup vs baseline: 2.0034x; 1.0817x over previous
"""EvolvedAttention Trainium2 Bass kernel (v2).

Full inputs -> full output. Sharding: 8 cores = 2 batches x 4 query-row
slices. Each core computes K/V/attention for its (batch, row-slice) with
all 16 heads; host slices inputs and concatenates row-slice outputs.

v2 design (from ntff trace of v1: DVE 73% busy on top-k counting, PE 38%
and cold):
  - Q/K/V projections in fp8e4 + DoubleRow (weights x32 host-side, folded
    back via Wo/32; cosine normalization cancels the scale for q/k).
  - gate/temp/Wo in fp16.
  - KnT (head-major [65, S], ones row for the threshold trick) and the
    gate stay SBUF-resident; no DRAM staging.
  - top-k threshold found on 4x-subsampled keys (strided matmul rhs),
    3 count-iterations split across ACT (Sign+accum), GPSIMD and DVE,
    bracketed false-position in "acc" space (acc = #ge - #lt).
  - scores recomputed transposed with threshold folded in (K=65), exp on
    ACT PSUM->fp8, mask on GPSIMD (em8 = [z>=0]*e8), AV in fp8 DoubleRow
    with a ones column in V8 for the softmax denominator.
  - selection of group g pipelines against attention of group g-1; the
    V projection fills the group-0 selection bubble.
"""

import os
import numpy as np

import concourse.bass as bass
import concourse.mybir as mybir
import concourse.tile as tile
from concourse import bacc

FP32 = mybir.dt.float32
FP16 = mybir.dt.float16
FP8 = mybir.dt.float8e4
U8 = mybir.dt.uint8
AF = mybir.ActivationFunctionType
ALU = mybir.AluOpType
DR = mybir.MatmulPerfMode.DoubleRow

WSCALE = 32.0


class Cfg:
    def __init__(self):
        self.S = 2048
        self.D = 1024
        self.NH = 16
        self.DH = 64
        self.RS = 512
        self.KK = self.S // 4          # top-k
        self.SUB = 4                   # key subsample for threshold search
        self.SS = self.S // self.SUB   # sampled keys (512)
        self.DCH = self.D // 128       # 8
        self.KC = self.S // 128        # 16
        self.RC = self.RS // 128       # 4
        self.HP = self.NH // 2         # 8
        self.GROUP = 4
        self.NG = self.NH // self.GROUP
        self.n_sel_iters = 3
        # target in acc space: acc = 2*c - SS, c target = KK/SUB
        self.ATGT = float(2 * (self.KK // self.SUB) - self.SS)  # -256
        self.slope0 = 2.0 * 2.8 * self.SS  # d(acc)/dt estimate


def build(cfg: Cfg, with_bias: bool):
    nc = bacc.Bacc()
    S, D, NH, DH, RS = cfg.S, cfg.D, cfg.NH, cfg.DH, cfg.RS
    DCH, KC, RC, HP = cfg.DCH, cfg.KC, cfg.RC, cfg.HP
    SS, G, NG = cfg.SS, cfg.GROUP, cfg.NG

    x8T = nc.dram_tensor("x8T", [128, DCH, S], FP8, kind="ExternalInput")
    x16T = nc.dram_tensor("x16T", [128, DCH, RS], FP16, kind="ExternalInput")
    xs = nc.dram_tensor("xs", [128, RC, D], FP32, kind="ExternalInput")
    Wq = nc.dram_tensor("Wq", [128, DCH, D], FP8, kind="ExternalInput")
    Wk = nc.dram_tensor("Wk", [128, DCH, D], FP8, kind="ExternalInput")
    Wv = nc.dram_tensor("Wv", [128, DCH, D], FP8, kind="ExternalInput")
    Wg = nc.dram_tensor("Wg", [128, DCH, D], FP16, kind="ExternalInput")
    Wo = nc.dram_tensor("Wo", [128, HP, D], FP16, kind="ExternalInput")
    Wt = nc.dram_tensor("Wt", [128, DCH], FP8, kind="ExternalInput")
    bt = nc.dram_tensor("bt", [1, 1], FP32, kind="ExternalInput")
    if with_bias:
        bq = nc.dram_tensor("bq", [1, D], FP16, kind="ExternalInput")
        bk = nc.dram_tensor("bk", [1, D], FP16, kind="ExternalInput")
        bv = nc.dram_tensor("bv", [1, D], FP16, kind="ExternalInput")
        bg = nc.dram_tensor("bg", [1, D], FP16, kind="ExternalInput")
        bo = nc.dram_tensor("bo", [1, D], FP16, kind="ExternalInput")
    out = nc.dram_tensor("out", [128, RC, D], FP32, kind="ExternalOutput")

    with tile.TileContext(nc) as tc:
        with (
            tc.tile_pool(name="persist", bufs=1) as pp,
            tc.tile_pool(name="psum", bufs=2, space="PSUM") as ps,
        ):
            # ---------------- persistent tiles (phase A) ----------------
            ident = pp.tile([128, 128], FP16, tag="ident")
            from concourse.masks import make_identity
            make_identity(nc, ident[:])
            ones_h = pp.tile([1, 128], FP16, tag="ones_h")
            nc.vector.memset(ones_h[:], 1.0)
            KnT = [pp.tile([65, S], FP16, tag=f"knt{h}", name=f"knt{h}")
                   for h in range(NH)]
            QnT = [pp.tile([65, RS], FP16, tag=f"qnt{h}", name=f"qnt{h}")
                   for h in range(NH)]
            gate16 = pp.tile([128, RC, D], FP16, tag="gate16")
            invt128 = pp.tile([128, 1], FP32, tag="invt128")
            bt_t = pp.tile([1, 1], FP32, tag="bt")
            nc.sync.dma_start(bt_t[:], bt[:])
            wt_t = pp.tile([128, DCH], FP8, tag="wt")
            nc.sync.dma_start(wt_t[:], Wt[:])
            bias_t = {}
            if with_bias:
                for nm, dram in (("bq", bq), ("bk", bk), ("bv", bv),
                                 ("bg", bg), ("bo", bo)):
                    t = pp.tile([1, D], FP16, tag=nm, name=f"b_{nm}")
                    nc.sync.dma_start(t[:], dram[:])
                    bias_t[nm] = t

            def pt1024(name):
                """projection psum: [128,1024] (2 banks), ring of 2."""
                return ps.tile([128, 1024], FP32, tag="pt", bufs=2,
                               padded_shape=[128, 1024], name=name)

            def ps512(name, shape=None, dtype=FP32):
                """small psum ring (transposes, sel-scores, gate, temp)."""
                return ps.tile(shape or [128, 512], dtype, tag="tps",
                               bufs=2, padded_shape=[128, 512], name=name)

            # ---------------- helpers ----------------
            def proj_fp8(xt8, w_dram, bias_row, n_chunks, wpool, wtag):
                """fp8 DoubleRow projection; yields (j, pt) with pt a
                [128,1024] psum row-chunk."""
                w = wpool.tile([128, DCH, D], FP8, tag=wtag, name=wtag,
                               bufs=2)
                nc.sync.dma_start(w[:], w_dram[:])
                for j in range(n_chunks):
                    pt = pt1024(f"pt_{wtag}")
                    for n in range(2):
                        sl = slice(n * 512, (n + 1) * 512)
                        for cp in range(DCH // 2):
                            nc.tensor.matmul(
                                pt[:, sl],
                                xt8[:, 2 * cp : 2 * cp + 2,
                                    j * 128 : (j + 1) * 128],
                                w[:, 2 * cp : 2 * cp + 2, sl],
                                start=(cp == 0),
                                stop=(cp == DCH // 2 - 1 and bias_row is None),
                                perf_mode=DR)
                        if bias_row is not None:
                            nc.tensor.matmul(
                                pt[:, sl], ones_h[:], bias_row[:, sl],
                                start=False, stop=True)
                    yield j, pt

            def proj_fp16_half(xt16, w_dram, bias_row, n, n_chunks, wpool,
                               wtag):
                """one 512-wide output half of an fp16 projection; yields
                (j, pt) psum tiles [128,512]."""
                w = wpool.tile([128, DCH, 512], FP16, tag=wtag, name=wtag,
                               bufs=2)
                nc.sync.dma_start(w[:], w_dram[:, :, n * 512 : (n + 1) * 512])
                for j in range(n_chunks):
                    pt = ps512(f"pt_{wtag}{n}")
                    for c in range(DCH):
                        nc.tensor.matmul(
                            pt[:],
                            xt16[:, c, j * 128 : (j + 1) * 128],
                            w[:, c, :],
                            start=(c == 0),
                            stop=(c == DCH - 1 and bias_row is None))
                    if bias_row is not None:
                        nc.tensor.matmul(
                            pt[:], ones_h[:],
                            bias_row[:, n * 512 : (n + 1) * 512],
                            start=False, stop=True)
                    yield j, pt

            def normalize_pair(sp, pt, dst16, extra_scale_ap):
                """cosine-normalize a [128,1024] psum row-chunk into
                dst16 [128, D] fp16."""
                sq = sp.tile([128, D], FP16, tag="sq", name="sq", bufs=3)
                nc.scalar.activation(sq[:], pt[:], AF.Square)
                n2 = sp.tile([128, NH], FP32, tag="n2", name="n2", bufs=3)
                nc.vector.tensor_reduce(
                    n2[:], sq[:].rearrange("p (h d) -> p h d", h=NH),
                    axis=mybir.AxisListType.X, op=ALU.add)
                rec = sp.tile([128, NH], FP32, tag="rec", name="rec", bufs=3)
                nc.vector.tensor_scalar_max(rec[:], n2[:], 1e-12)
                nc.vector.reciprocal(rec[:], rec[:])
                rsq = sp.tile([128, NH], FP32, tag="rsq", name="rsq", bufs=3)
                nc.scalar.activation(rsq[:], rec[:], AF.Sqrt)
                if extra_scale_ap is not None:
                    nc.vector.tensor_scalar(
                        out=rsq[:], in0=rsq[:], scalar1=extra_scale_ap,
                        scalar2=None, op0=ALU.mult)
                nc.vector.tensor_tensor(
                    dst16[:].rearrange("p (h d) -> p h d", h=NH),
                    pt[:].rearrange("p (h d) -> p h d", h=NH),
                    rsq[:].rearrange("p (h o) -> p h o", o=1)
                        .to_broadcast([128, NH, DH]),
                    ALU.mult)

            def transpose_to_heads(dst_of_head, src16, j, who):
                """src16 [128 rows, 1024] -> per-head [64, 128] blocks into
                dst_of_head(h)[0:64, j*128:(j+1)*128]."""
                for p in range(HP):
                    tps = ps.tile([128, 128], FP16, tag="tps", bufs=2,
                                  padded_shape=[128, 512], name=f"tps_{who}")
                    nc.tensor.transpose(
                        tps[:], src16[:, p * 128 : (p + 1) * 128], ident[:])
                    for hh in range(2):
                        h = 2 * p + hh
                        dst = dst_of_head(h)[0:64, j * 128 : (j + 1) * 128]
                        src = tps[hh * 64 : hh * 64 + 64, :]
                        if (p + hh + j) % 2 == 0:
                            nc.scalar.activation(dst, src, AF.Copy)
                        else:
                            nc.vector.tensor_copy(dst, src)

            # ================ phase A ================
            with tc.tile_pool(name="poolX8", bufs=1) as px:
                xt8 = px.tile([128, DCH, S], FP8, tag="xt8")
                nc.sync.dma_start(xt8[:], x8T[:])

                with (
                    tc.tile_pool(name="poolA", bufs=1) as pa,
                    tc.tile_pool(name="wpoolA", bufs=2) as wpa,
                ):
                    xt16 = pa.tile([128, DCH, RS], FP16, tag="xt16")
                    nc.sync.dma_start(xt16[:], x16T[:])

                    # --- K projection -> KnT (resident) ---
                    for j, pt in proj_fp8(xt8, Wk, bias_t.get("bk"),
                                          KC, wpa, "w8"):
                        kn = pa.tile([128, D], FP16, tag="kn", name="kn",
                                     bufs=3)
                        normalize_pair(pa, pt, kn, None)
                        transpose_to_heads(lambda h: KnT[h], kn, j, "k")

                    # --- temp (from fp8 x; scale folded into sigmoid) ---
                    tp = ps.tile([1, 512], FP32, tag="tps", bufs=2,
                                 padded_shape=[128, 512], name="tp_temp")
                    first = True
                    for c in range(DCH):
                        for j in range(4):
                            nc.tensor.matmul(
                                tp[:], wt_t[:, c : c + 1],
                                xt8[:, c, j * 512 : (j + 1) * 512],
                                start=first,
                                stop=(c == DCH - 1 and j == 3))
                            first = False
                    tsum = pa.tile([1, 1], FP32, tag="tsum")
                    nc.vector.tensor_reduce(tsum[:], tp[:],
                                            axis=mybir.AxisListType.X,
                                            op=ALU.add)
                    sig = pa.tile([1, 1], FP32, tag="sig")
                    nc.scalar.activation(sig[:], tsum[:], AF.Sigmoid,
                                         bias=bt_t[:],
                                         scale=1.0 / (S * WSCALE))
                    temp = pa.tile([1, 1], FP32, tag="temp")
                    nc.vector.tensor_scalar_add(temp[:], sig[:], 0.5)
                    invt = pa.tile([1, 1], FP32, tag="invt")
                    nc.vector.reciprocal(invt[:], temp[:])
                    nc.gpsimd.partition_broadcast(invt128[:], invt[:])

                    # --- Q projection -> QnT (1/temp folded in) ---
                    for j, pt in proj_fp8(xt8, Wq, bias_t.get("bq"),
                                          RC, wpa, "w8"):
                        qn = pa.tile([128, D], FP16, tag="qn", name="qn",
                                     bufs=3)
                        normalize_pair(pa, pt, qn, invt128[:, 0:1])
                        transpose_to_heads(lambda h: QnT[h], qn, j, "q")

                    # --- gate (fp16, query slice only, resident) ---
                    for n in range(2):
                        for j, pt in proj_fp16_half(
                                xt16, Wg, bias_t.get("bg"), n, RC, wpa,
                                "wg16h"):
                            nc.scalar.activation(
                                gate16[:, j, n * 512 : (n + 1) * 512],
                                pt[:], AF.Sigmoid)

                # ---- late persistent tiles (group phase) ----
                V8 = pp.tile([128, KC, NH, 66], FP8, tag="v8")
                nc.vector.memset(V8[:, :, :, 64:66], 1.0)
                attnT = pp.tile([128, HP, RS], FP16, tag="attnT")
                for h in range(NH):
                    nc.gpsimd.memset(KnT[h][64:65, :], 1.0)

                # ============ selection / attention bodies ============
                def selection(gi, gp):
                    heads = list(range(gi * G, (gi + 1) * G))
                    nt = G * RC
                    nt0 = gp.tile([128, 1], FP32, tag="nt0")
                    nc.vector.memset(nt0[:], -0.1)
                    st_t = gp.tile([128, nt], FP32, tag="st_t")
                    st_lo = gp.tile([128, nt], FP32, tag="st_lo")
                    st_hi = gp.tile([128, nt], FP32, tag="st_hi")
                    st_clo = gp.tile([128, nt], FP32, tag="st_clo")
                    st_chi = gp.tile([128, nt], FP32, tag="st_chi")
                    acc = gp.tile([128, nt], FP32, tag="acc")
                    nc.vector.memset(st_t[:], 0.1)
                    nc.vector.memset(st_lo[:], -2.1)
                    nc.vector.memset(st_hi[:], 2.1)
                    nc.vector.memset(st_clo[:], float(SS))
                    nc.vector.memset(st_chi[:], float(-SS))

                    # selection scores (subsampled keys, strided rhs)
                    s16 = {}
                    for hi_, h in enumerate(heads):
                        for j in range(RC):
                            sp_ = ps512(f"selp_{hi_}_{j}")
                            nc.tensor.matmul(
                                sp_[:],
                                QnT[h][0:64, j * 128 : (j + 1) * 128],
                                KnT[h][0:64, 1 : S : cfg.SUB],
                                start=True, stop=True)
                            srow = gp.tile([128, SS], FP16,
                                           tag=f"s16_{hi_}_{j}",
                                           name=f"s16_{hi_}_{j}")
                            if (hi_ + j) % 2 == 0:
                                nc.scalar.activation(srow[:], sp_[:],
                                                     AF.Copy)
                            else:
                                nc.vector.tensor_copy(srow[:], sp_[:])
                            s16[(hi_, j)] = srow

                    negt = gp.tile([128, nt], FP32, tag="negt")
                    for it in range(cfg.n_sel_iters):
                        use_act = (it != 1)
                        if it == 2:
                            nc.vector.tensor_scalar(
                                out=negt[:], in0=st_t[:], scalar1=-1.0,
                                scalar2=None, op0=ALU.mult)
                        for hi_, h in enumerate(heads):
                            for j in range(RC):
                                col = hi_ * RC + j
                                if use_act:
                                    bias_ap = (nt0[:, 0:1] if it == 0
                                               else negt[:, col : col + 1])
                                    scr = gp.tile([128, SS], FP8, tag="scr8",
                                                  bufs=2, name="scr8")
                                    nc.scalar.activation(
                                        scr[:], s16[(hi_, j)][:], AF.Sign,
                                        bias=bias_ap,
                                        accum_out=acc[:, col : col + 1])
                                else:
                                    scr = gp.tile([128, SS], FP16,
                                                  tag="scr16", bufs=2,
                                                  name="scr16")
                                    nc.vector.tensor_scalar(
                                        out=scr[:], in0=s16[(hi_, j)][:],
                                        scalar1=st_t[:, col : col + 1],
                                        scalar2=None, op0=ALU.is_ge,
                                        op1=ALU.add,
                                        accum_out=acc[:, col : col + 1])
                        if not use_act:
                            # c in [0,SS] -> acc = 2c - SS
                            nc.vector.tensor_scalar(
                                out=acc[:], in0=acc[:], scalar1=2.0,
                                scalar2=float(-SS), op0=ALU.mult,
                                op1=ALU.add)
                        islo = gp.tile([128, nt], U8, tag="islo", bufs=2)
                        nc.vector.tensor_scalar(
                            out=islo[:], in0=acc[:], scalar1=cfg.ATGT,
                            scalar2=None, op0=ALU.is_ge)
                        nc.vector.copy_predicated(st_lo[:], islo[:], st_t[:])
                        nc.vector.copy_predicated(st_clo[:], islo[:], acc[:])
                        ishi = gp.tile([128, nt], U8, tag="ishi", bufs=2)
                        nc.vector.tensor_scalar(
                            out=ishi[:], in0=acc[:], scalar1=cfg.ATGT,
                            scalar2=None, op0=ALU.is_lt)
                        nc.vector.copy_predicated(st_hi[:], ishi[:], st_t[:])
                        nc.vector.copy_predicated(st_chi[:], ishi[:], acc[:])
                        tnew = gp.tile([128, nt], FP32, tag="tnew", bufs=2)
                        if it == 0:
                            nc.vector.tensor_scalar(
                                out=tnew[:], in0=acc[:], scalar1=cfg.ATGT,
                                scalar2=1.0 / cfg.slope0, op0=ALU.subtract,
                                op1=ALU.mult)
                            nc.vector.tensor_add(tnew[:], tnew[:], st_t[:])
                        else:
                            den = gp.tile([128, nt], FP32, tag="den",
                                          bufs=2)
                            nc.vector.tensor_sub(den[:], st_clo[:],
                                                 st_chi[:])
                            nc.vector.tensor_scalar_max(den[:], den[:], 1.0)
                            rden = gp.tile([128, nt], FP32, tag="rden",
                                           bufs=2)
                            nc.vector.reciprocal(rden[:], den[:])
                            nc.vector.tensor_scalar(
                                out=tnew[:], in0=st_clo[:],
                                scalar1=cfg.ATGT,
                                scalar2=None, op0=ALU.subtract)
                            nc.vector.tensor_mul(tnew[:], tnew[:], rden[:])
                            wid = gp.tile([128, nt], FP32, tag="wid",
                                          bufs=2)
                            nc.vector.tensor_sub(wid[:], st_hi[:], st_lo[:])
                            nc.vector.tensor_mul(tnew[:], tnew[:], wid[:])
                            nc.vector.tensor_add(tnew[:], tnew[:], st_lo[:])
                        nc.vector.tensor_tensor(tnew[:], tnew[:], st_lo[:],
                                                ALU.max)
                        nc.vector.tensor_tensor(tnew[:], tnew[:], st_hi[:],
                                                ALU.min)
                        iseq = gp.tile([128, nt], U8, tag="iseq", bufs=2)
                        nc.vector.tensor_scalar(
                            out=iseq[:], in0=acc[:], scalar1=cfg.ATGT,
                            scalar2=None, op0=ALU.not_equal)
                        nc.vector.copy_predicated(st_t[:], iseq[:], tnew[:])

                    # tneg = -t (scores already in /temp units); write into
                    # QnT[h] row 64 via PE transpose + small copies
                    tneg = gp.tile([128, nt], FP16, tag="tneg")
                    nc.vector.tensor_scalar(
                        out=tneg[:], in0=st_t[:], scalar1=-1.0,
                        scalar2=None, op0=ALU.mult)
                    ttp = ps.tile([nt, 128], FP16, tag="tps", bufs=2,
                                  padded_shape=[128, 512], name="ttp")
                    nc.tensor.transpose(ttp[:], tneg[:], ident[:])
                    tnT = gp.tile([nt, 128], FP16, tag="tnT")
                    nc.scalar.activation(tnT[:], ttp[:], AF.Copy)
                    for hi_, h in enumerate(heads):
                        for j in range(RC):
                            col = hi_ * RC + j
                            nc.sync.dma_start(
                                QnT[h][64:65, j * 128 : (j + 1) * 128],
                                tnT[col : col + 1, :])

                def attention(gi):
                    heads = list(range(gi * G, (gi + 1) * G))
                    for hi_, h in enumerate(heads):
                        avp = ps.tile([65, RS], FP32, tag="avp", bufs=2,
                                      padded_shape=[128, 512], name="avp")
                        for kcp in range(KC // 2):
                            em8 = pp.tile([128, 2, RS], FP8, tag="em8",
                                          bufs=4, name="em8")
                            stp = ps.tile([128, 2, RS], FP32, tag="pt",
                                          bufs=2,
                                          padded_shape=[128, 2, 512],
                                          name="stp")
                            for sub in range(2):
                                kc = 2 * kcp + sub
                                nc.tensor.matmul(
                                    stp[:, sub, :],
                                    KnT[h][:, kc * 128 : (kc + 1) * 128],
                                    QnT[h][:], start=True, stop=True)
                            e16 = pp.tile([128, 2, RS], FP16, tag="e16",
                                          bufs=2, name="e16")
                            nc.scalar.activation(e16[:], stp[:], AF.Exp)
                            nc.vector.scalar_tensor_tensor(
                                out=em8[:], in0=e16[:],
                                scalar=1.0, in1=e16[:],
                                op0=ALU.is_ge, op1=ALU.mult)
                            nc.tensor.matmul(
                                avp[:],
                                V8[:, 2 * kcp : 2 * kcp + 2, h, 0:65],
                                em8[:, :, :],
                                start=(kcp == 0), stop=(kcp == KC // 2 - 1),
                                perf_mode=DR)
                        # normalize: attnT = avp[0:64] * (1/z)
                        zrow = pp.tile([1, RS], FP32, tag="zrow", bufs=2,
                                       name="zrow")
                        nc.vector.tensor_scalar_max(zrow[:], avp[64:65, :],
                                                    1e-30)
                        zrec = pp.tile([1, RS], FP32, tag="zrec", bufs=2,
                                       name="zrec")
                        nc.vector.reciprocal(zrec[:], zrow[:])
                        zrep = pp.tile([64, RS], FP32, tag="zrep", bufs=2,
                                       name="zrep")
                        nc.gpsimd.partition_broadcast(zrep[:], zrec[:])
                        nc.vector.tensor_tensor(
                            attnT[(h % 2) * 64 : (h % 2) * 64 + 64,
                                  h // 2, :],
                            avp[0:64, :], zrep[:], ALU.mult)

                # ============ pipeline: selection(g) | attention(g-1) ====
                with tc.tile_pool(name="poolG0", bufs=1) as gp0:
                    selection(0, gp0)
                with tc.tile_pool(name="poolV", bufs=1) as pv:
                    for j, pt in proj_fp8(xt8, Wv, bias_t.get("bv"),
                                          KC, pv, "wv8"):
                        dst = V8[:, j, :, 0:64]
                        src = pt[:].rearrange("p (h d) -> p h d", h=NH)
                        if j % 2 == 0:
                            nc.scalar.activation(dst, src, AF.Copy)
                        else:
                            nc.vector.tensor_copy(dst, src)

            # poolX8 closed (xt8 freed)
            for gi in range(1, NG):
                with tc.tile_pool(name=f"poolG{gi}", bufs=1) as gp_:
                    selection(gi, gp_)
                attention(gi - 1)
            attention(NG - 1)

            # ================ phase C: out proj + gate ================
            with tc.tile_pool(name="poolC", bufs=1) as pc:
                wo_t = pc.tile([128, HP, D], FP16, tag="wo")
                nc.sync.dma_start(wo_t[:], Wo[:])
                xs_t = pc.tile([128, RC, D], FP32, tag="xs")
                nc.sync.dma_start(xs_t[:], xs[:])
                for j in range(RC):
                    op = pt1024("op_out")
                    for n in range(2):
                        sl = slice(n * 512, (n + 1) * 512)
                        for p in range(HP):
                            nc.tensor.matmul(
                                op[:, sl],
                                attnT[:, p, j * 128 : (j + 1) * 128],
                                wo_t[:, p, sl],
                                start=(p == 0),
                                stop=(p == HP - 1 and not with_bias))
                        if with_bias:
                            nc.tensor.matmul(
                                op[:, sl], ones_h[:], bias_t["bo"][:, sl],
                                start=False, stop=True)
                    dd = pc.tile([128, D], FP32, tag="dd", bufs=2,
                                 name="dd")
                    nc.vector.tensor_sub(dd[:], op[:], xs_t[:, j, :])
                    nc.vector.tensor_mul(dd[:], dd[:], gate16[:, j, :])
                    oo = pc.tile([128, D], FP32, tag="oo", bufs=2,
                                 name="oo")
                    nc.gpsimd.tensor_add(oo[:], dd[:], xs_t[:, j, :])
                    nc.sync.dma_start(out[:, j, :], oo[:])

    nc.finalize()
    return nc


# ---------------------------------------------------------------------------
_NC_CACHE = {}
LAST_EXEC_NS = None
LAST_RESULTS = None


def _get_nc(with_bias: bool):
    key = bool(with_bias)
    if key not in _NC_CACHE:
        _NC_CACHE[key] = build(Cfg(), key)
    return _NC_CACHE[key]


def _pack_core_inputs(x, Wq, bq, Wk, bk, Wv, bv, Wo, bo, Wt, bt, Wg, bg,
                      b, r0, cfg, with_bias, fp8):
    S, D, RS, DCH, HP = cfg.S, cfg.D, cfg.RS, cfg.DCH, cfg.HP
    xb = x[b]
    xt = np.ascontiguousarray(
        np.roll(xb.T, -r0, axis=1).reshape(DCH, 128, S).transpose(1, 0, 2))
    xss = np.ascontiguousarray(
        xb[r0 : r0 + RS].reshape(cfg.RC, 128, D).transpose(1, 0, 2))

    def wpack(W, dt, scale=1.0):
        return np.ascontiguousarray(
            (W * scale).reshape(DCH, 128, D).transpose(1, 0, 2)).astype(dt)

    m = {
        "x8T": xt.astype(fp8),
        "x16T": np.ascontiguousarray(xt[:, :, 0:RS]).astype(np.float16),
        "xs": xss.astype(np.float32),
        "Wq": wpack(Wq, fp8, WSCALE),
        "Wk": wpack(Wk, fp8, WSCALE),
        "Wv": wpack(Wv, fp8, WSCALE),
        "Wg": wpack(Wg, np.float16),
        "Wo": np.ascontiguousarray(
            (Wo / WSCALE).reshape(HP, 128, D).transpose(1, 0, 2))
            .astype(np.float16),
        "Wt": np.ascontiguousarray(
            Wt.reshape(DCH, 128).T * WSCALE).astype(fp8),
        "bt": bt.reshape(1, 1).astype(np.float32),
    }
    if with_bias:
        m["bq"] = (bq * WSCALE).reshape(1, D).astype(np.float16)
        m["bk"] = (bk * WSCALE).reshape(1, D).astype(np.float16)
        m["bv"] = (bv * WSCALE).reshape(1, D).astype(np.float16)
        m["bg"] = bg.reshape(1, D).astype(np.float16)
        m["bo"] = bo.reshape(1, D).astype(np.float16)
    return m


def kernel(**inputs):
    from concourse.bass_utils import run_bass_kernel_spmd
    cfg = Cfg()
    fp8 = mybir.dt.np(FP8)
    x = np.asarray(inputs["x"], np.float32)
    B, S, D = x.shape
    args = [np.asarray(inputs[k]) for k in
            ("Wq", "bq", "Wk", "bk", "Wv", "bv", "Wo", "bo", "Wt", "bt",
             "Wg", "bg")]
    with_bias = any(np.any(np.asarray(inputs[k])) for k in
                    ("bq", "bk", "bv", "bg", "bo"))
    nc = _get_nc(with_bias)
    in_maps = []
    for c in range(8):
        b, q = c // 4, c % 4
        in_maps.append(_pack_core_inputs(
            x, *args, b, q * cfg.RS, cfg, with_bias, fp8))
    trace = bool(int(os.environ.get("KERNEL_TRACE", "0")))
    res = run_bass_kernel_spmd(nc, in_maps, core_ids=list(range(8)),
                               trace=trace)
    global LAST_EXEC_NS, LAST_RESULTS
    LAST_EXEC_NS = res.exec_time_ns
    LAST_RESULTS = res
    out = np.empty((B, S, D), np.float32)
    for c in range(8):
        b, q = c // 4, c % 4
        o = res.results[c]["out"]  # [128, RC, D]
        out[b, q * cfg.RS : (q + 1) * cfg.RS] = \
            o.transpose(1, 0, 2).reshape(cfg.RS, D)
    return out


# revision 48
# speedup vs baseline: 2.2706x; 1.1334x over previous
"""EvolvedAttention Trainium2 Bass kernel (v2).

Full inputs -> full output. Sharding: 8 cores = 2 batches x 4 query-row
slices. Each core computes K/V/attention for its (batch, row-slice) with
all 16 heads; host slices inputs and concatenates row-slice outputs.

v2 design (from ntff trace of v1: DVE 73% busy on top-k counting, PE 38%
and cold):
  - Q/K/V projections in fp8e4 + DoubleRow (weights x32 host-side, folded
    back via Wo/32; cosine normalization cancels the scale for q/k).
  - gate/temp/Wo in fp16.
  - KnT (head-major [65, S], ones row for the threshold trick) and the
    gate stay SBUF-resident; no DRAM staging.
  - top-k threshold found on 4x-subsampled keys (strided matmul rhs),
    3 count-iterations split across ACT (Sign+accum), GPSIMD and DVE,
    bracketed false-position in "acc" space (acc = #ge - #lt).
  - scores recomputed transposed with threshold folded in (K=65), exp on
    ACT PSUM->fp8, mask on GPSIMD (em8 = [z>=0]*e8), AV in fp8 DoubleRow
    with a ones column in V8 for the softmax denominator.
  - selection of group g pipelines against attention of group g-1; the
    V projection fills the group-0 selection bubble.
"""

import os
import numpy as np

import concourse.bass as bass
import concourse.mybir as mybir
import concourse.tile as tile
from concourse import bacc

FP32 = mybir.dt.float32
FP16 = mybir.dt.float16
FP8 = mybir.dt.float8e4
U8 = mybir.dt.uint8
AF = mybir.ActivationFunctionType
ALU = mybir.AluOpType
DR = mybir.MatmulPerfMode.DoubleRow

WSCALE = 32.0


class Cfg:
    def __init__(self):
        self.S = 2048
        self.D = 1024
        self.NH = 16
        self.DH = 64
        self.RS = 512
        self.KK = self.S // 4          # top-k
        self.SUB = 4                   # key subsample for threshold search
        self.SS = self.S // self.SUB   # sampled keys (512)
        self.DCH = self.D // 128       # 8
        self.KC = self.S // 128        # 16
        self.RC = self.RS // 128       # 4
        self.HP = self.NH // 2         # 8
        self.GROUP = 4
        self.NG = self.NH // self.GROUP
        self.n_sel_iters = 2
        # target in acc space: acc = 2*c - SS, c target = KK/SUB
        self.ATGT = float(2 * (self.KK // self.SUB) - self.SS)  # -256
        self.slope0 = 2.0 * 2.8 * self.SS  # d(acc)/dt estimate


def build(cfg: Cfg, with_bias: bool):
    nc = bacc.Bacc()
    S, D, NH, DH, RS = cfg.S, cfg.D, cfg.NH, cfg.DH, cfg.RS
    DCH, KC, RC, HP = cfg.DCH, cfg.KC, cfg.RC, cfg.HP
    SS, G, NG = cfg.SS, cfg.GROUP, cfg.NG

    x8T = nc.dram_tensor("x8T", [128, DCH, S], FP8, kind="ExternalInput")
    x16T = nc.dram_tensor("x16T", [128, DCH, RS], FP16, kind="ExternalInput")
    xs = nc.dram_tensor("xs", [128, RC, D], FP32, kind="ExternalInput")
    Wq = nc.dram_tensor("Wq", [128, DCH, D], FP8, kind="ExternalInput")
    Wk = nc.dram_tensor("Wk", [128, DCH, D], FP8, kind="ExternalInput")
    Wv = nc.dram_tensor("Wv", [128, DCH, D], FP8, kind="ExternalInput")
    Wg = nc.dram_tensor("Wg", [128, DCH, D], FP16, kind="ExternalInput")
    Wo = nc.dram_tensor("Wo", [128, HP, D], FP16, kind="ExternalInput")
    Wt = nc.dram_tensor("Wt", [128, DCH], FP8, kind="ExternalInput")
    bt = nc.dram_tensor("bt", [1, 1], FP32, kind="ExternalInput")
    if with_bias:
        bq = nc.dram_tensor("bq", [1, D], FP16, kind="ExternalInput")
        bk = nc.dram_tensor("bk", [1, D], FP16, kind="ExternalInput")
        bv = nc.dram_tensor("bv", [1, D], FP16, kind="ExternalInput")
        bg = nc.dram_tensor("bg", [1, D], FP16, kind="ExternalInput")
        bo = nc.dram_tensor("bo", [1, D], FP16, kind="ExternalInput")
    out = nc.dram_tensor("out", [128, RC, D], FP32, kind="ExternalOutput")

    with tile.TileContext(nc) as tc:
        with (
            tc.tile_pool(name="persist", bufs=1) as pp,
            tc.tile_pool(name="psum", bufs=2, space="PSUM") as ps,
        ):
            # ---------------- persistent tiles (phase A) ----------------
            ident = pp.tile([128, 128], FP16, tag="ident")
            from concourse.masks import make_identity
            make_identity(nc, ident[:])
            ones_h = pp.tile([1, 128], FP16, tag="ones_h")
            nc.vector.memset(ones_h[:], 1.0)
            KnT = [pp.tile([65, S], FP16, tag=f"knt{h}", name=f"knt{h}")
                   for h in range(NH)]
            QnT = [pp.tile([65, RS], FP16, tag=f"qnt{h}", name=f"qnt{h}")
                   for h in range(NH)]
            for h in range(NH):
                nc.gpsimd.memset(KnT[h][64:65, :], 1.0)
            gate16 = pp.tile([128, RC, D], FP16, tag="gate16")
            invt128 = pp.tile([128, 1], FP32, tag="invt128")
            bt_t = pp.tile([1, 1], FP32, tag="bt")
            nc.sync.dma_start(bt_t[:], bt[:])
            wt_t = pp.tile([128, DCH], FP8, tag="wt")
            nc.sync.dma_start(wt_t[:], Wt[:])
            bias_t = {}
            if with_bias:
                for nm, dram in (("bq", bq), ("bk", bk), ("bv", bv),
                                 ("bg", bg), ("bo", bo)):
                    t = pp.tile([1, D], FP16, tag=nm, name=f"b_{nm}")
                    nc.sync.dma_start(t[:], dram[:])
                    bias_t[nm] = t

            def pt1024(name):
                """projection psum: [128,1024] (2 banks), ring of 2."""
                return ps.tile([128, 1024], FP32, tag="pt", bufs=2,
                               padded_shape=[128, 1024], name=name)

            def ps512(name, shape=None, dtype=FP32):
                """small psum ring (transposes, sel-scores, gate, temp)."""
                return ps.tile(shape or [128, 512], dtype, tag="tps",
                               bufs=2, padded_shape=[128, 512], name=name)

            # ---------------- helpers ----------------
            def proj_fp8(xt8, w_dram, bias_row, n_chunks, wpool, wtag):
                """fp8 DoubleRow projection; yields (j, pt) with pt a
                [128,1024] psum row-chunk."""
                w = wpool.tile([128, DCH, D], FP8, tag=wtag, name=wtag,
                               bufs=2)
                nc.sync.dma_start(w[:], w_dram[:])
                for j in range(n_chunks):
                    pt = pt1024(f"pt_{wtag}")
                    for n in range(2):
                        sl = slice(n * 512, (n + 1) * 512)
                        for cp in range(DCH // 2):
                            nc.tensor.matmul(
                                pt[:, sl],
                                xt8[:, 2 * cp : 2 * cp + 2,
                                    j * 128 : (j + 1) * 128],
                                w[:, 2 * cp : 2 * cp + 2, sl],
                                start=(cp == 0),
                                stop=(cp == DCH // 2 - 1 and bias_row is None),
                                perf_mode=DR)
                        if bias_row is not None:
                            nc.tensor.matmul(
                                pt[:, sl], ones_h[:], bias_row[:, sl],
                                start=False, stop=True)
                    yield j, pt

            def proj_fp16_half(xt16, w_dram, bias_row, n, n_chunks, wpool,
                               wtag):
                """one 512-wide output half of an fp16 projection; yields
                (j, pt) psum tiles [128,512]."""
                w = wpool.tile([128, DCH, 512], FP16, tag=wtag, name=wtag,
                               bufs=2)
                nc.sync.dma_start(w[:], w_dram[:, :, n * 512 : (n + 1) * 512])
                for j in range(n_chunks):
                    pt = ps512(f"pt_{wtag}{n}")
                    for c in range(DCH):
                        nc.tensor.matmul(
                            pt[:],
                            xt16[:, c, j * 128 : (j + 1) * 128],
                            w[:, c, :],
                            start=(c == 0),
                            stop=(c == DCH - 1 and bias_row is None))
                    if bias_row is not None:
                        nc.tensor.matmul(
                            pt[:], ones_h[:],
                            bias_row[:, n * 512 : (n + 1) * 512],
                            start=False, stop=True)
                    yield j, pt

            def normalize_pair(sp, pt, dst16, extra_scale_ap):
                """cosine-normalize a [128,1024] psum row-chunk into
                dst16 [128, D] fp16."""
                sq = sp.tile([128, D], FP16, tag="sq", name="sq", bufs=3)
                nc.scalar.activation(sq[:], pt[:], AF.Square)
                n2 = sp.tile([128, NH], FP32, tag="n2", name="n2", bufs=3)
                nc.vector.tensor_reduce(
                    n2[:], sq[:].rearrange("p (h d) -> p h d", h=NH),
                    axis=mybir.AxisListType.X, op=ALU.add)
                rec = sp.tile([128, NH], FP32, tag="rec", name="rec", bufs=3)
                nc.vector.tensor_scalar_max(rec[:], n2[:], 1e-12)
                nc.vector.reciprocal(rec[:], rec[:])
                rsq = sp.tile([128, NH], FP32, tag="rsq", name="rsq", bufs=3)
                nc.scalar.activation(rsq[:], rec[:], AF.Sqrt)
                if extra_scale_ap is not None:
                    nc.vector.tensor_scalar(
                        out=rsq[:], in0=rsq[:], scalar1=extra_scale_ap,
                        scalar2=None, op0=ALU.mult)
                nc.vector.tensor_tensor(
                    dst16[:].rearrange("p (h d) -> p h d", h=NH),
                    pt[:].rearrange("p (h d) -> p h d", h=NH),
                    rsq[:].rearrange("p (h o) -> p h o", o=1)
                        .to_broadcast([128, NH, DH]),
                    ALU.mult)

            def transpose_to_heads(dst_of_head, src16, j, who):
                """src16 [128 rows, 1024] -> per-head [64, 128] blocks into
                dst_of_head(h)[0:64, j*128:(j+1)*128]."""
                for p in range(HP):
                    tps = ps.tile([128, 128], FP16, tag="tps", bufs=2,
                                  padded_shape=[128, 512], name=f"tps_{who}")
                    nc.tensor.transpose(
                        tps[:], src16[:, p * 128 : (p + 1) * 128], ident[:])
                    for hh in range(2):
                        h = 2 * p + hh
                        dst = dst_of_head(h)[0:64, j * 128 : (j + 1) * 128]
                        src = tps[hh * 64 : hh * 64 + 64, :]
                        if (p + hh + j) % 2 == 0:
                            nc.scalar.activation(dst, src, AF.Copy)
                        else:
                            nc.vector.tensor_copy(dst, src)

            # ================ phase A ================
            with tc.tile_pool(name="poolX8", bufs=1) as px:
                xt8 = px.tile([128, DCH, S], FP8, tag="xt8")
                nc.sync.dma_start(xt8[:], x8T[:])

                with (
                    tc.tile_pool(name="poolA", bufs=1) as pa,
                    tc.tile_pool(name="wpoolA", bufs=2) as wpa,
                ):
                    xt16 = pa.tile([128, DCH, RS], FP16, tag="xt16")
                    nc.sync.dma_start(xt16[:], x16T[:])

                    # --- K projection -> KnT (resident) ---
                    for j, pt in proj_fp8(xt8, Wk, bias_t.get("bk"),
                                          KC, wpa, "w8"):
                        kn = pa.tile([128, D], FP16, tag="kn", name="kn",
                                     bufs=3)
                        normalize_pair(pa, pt, kn, None)
                        transpose_to_heads(lambda h: KnT[h], kn, j, "k")

                    # --- temp (from fp8 x; scale folded into sigmoid) ---
                    tp = ps.tile([1, 512], FP32, tag="tps", bufs=2,
                                 padded_shape=[128, 512], name="tp_temp")
                    first = True
                    for c in range(DCH):
                        for j in range(4):
                            nc.tensor.matmul(
                                tp[:], wt_t[:, c : c + 1],
                                xt8[:, c, j * 512 : (j + 1) * 512],
                                start=first,
                                stop=(c == DCH - 1 and j == 3))
                            first = False
                    tsum = pa.tile([1, 1], FP32, tag="tsum")
                    nc.vector.tensor_reduce(tsum[:], tp[:],
                                            axis=mybir.AxisListType.X,
                                            op=ALU.add)
                    sig = pa.tile([1, 1], FP32, tag="sig")
                    nc.scalar.activation(sig[:], tsum[:], AF.Sigmoid,
                                         bias=bt_t[:],
                                         scale=1.0 / (S * WSCALE))
                    temp = pa.tile([1, 1], FP32, tag="temp")
                    nc.vector.tensor_scalar_add(temp[:], sig[:], 0.5)
                    invt = pa.tile([1, 1], FP32, tag="invt")
                    nc.vector.reciprocal(invt[:], temp[:])
                    nc.gpsimd.partition_broadcast(invt128[:], invt[:])

                    # --- Q projection -> QnT (1/temp folded in) ---
                    for j, pt in proj_fp8(xt8, Wq, bias_t.get("bq"),
                                          RC, wpa, "w8"):
                        qn = pa.tile([128, D], FP16, tag="qn", name="qn",
                                     bufs=3)
                        normalize_pair(pa, pt, qn, invt128[:, 0:1])
                        transpose_to_heads(lambda h: QnT[h], qn, j, "q")

                    # --- gate (fp16, query slice only, resident) ---
                    for n in range(2):
                        for j, pt in proj_fp16_half(
                                xt16, Wg, bias_t.get("bg"), n, RC, wpa,
                                "wg16h"):
                            nc.scalar.activation(
                                gate16[:, j, n * 512 : (n + 1) * 512],
                                pt[:], AF.Sigmoid)

                # ---- late persistent tiles (group phase) ----
                V8 = pp.tile([128, KC, NH, 66], FP8, tag="v8")
                nc.gpsimd.memset(V8[:, :, :, 64:66], 1.0)
                attnT = pp.tile([128, HP, RS], FP16, tag="attnT")

                # ============ selection / attention bodies ============
                def selection_stages(gi, gp):
                    """returns 4 issue-stage closures for group gi's
                    threshold search (2 count iterations)."""
                    heads = list(range(gi * G, (gi + 1) * G))
                    nt = G * RC
                    st = {}

                    def bracket_update(it):
                        acc, st_t = st["acc"], st["st_t"]
                        st_lo, st_hi = st["st_lo"], st["st_hi"]
                        st_clo, st_chi = st["st_clo"], st["st_chi"]
                        islo = gp.tile([128, nt], U8, tag="islo", bufs=2)
                        nc.vector.tensor_scalar(
                            out=islo[:], in0=acc[:], scalar1=cfg.ATGT,
                            scalar2=None, op0=ALU.is_ge)
                        nc.vector.copy_predicated(st_lo[:], islo[:], st_t[:])
                        nc.vector.copy_predicated(st_clo[:], islo[:], acc[:])
                        ishi = gp.tile([128, nt], U8, tag="ishi", bufs=2)
                        nc.vector.tensor_scalar(
                            out=ishi[:], in0=acc[:], scalar1=cfg.ATGT,
                            scalar2=None, op0=ALU.is_lt)
                        nc.vector.copy_predicated(st_hi[:], ishi[:], st_t[:])
                        nc.vector.copy_predicated(st_chi[:], ishi[:], acc[:])
                        tnew = gp.tile([128, nt], FP32, tag="tnew", bufs=2)
                        if it == 0:
                            nc.vector.tensor_scalar(
                                out=tnew[:], in0=acc[:], scalar1=cfg.ATGT,
                                scalar2=1.0 / cfg.slope0, op0=ALU.subtract,
                                op1=ALU.mult)
                            nc.vector.tensor_add(tnew[:], tnew[:], st_t[:])
                        else:
                            den = gp.tile([128, nt], FP32, tag="den",
                                          bufs=2)
                            nc.vector.tensor_sub(den[:], st_clo[:],
                                                 st_chi[:])
                            nc.vector.tensor_scalar_max(den[:], den[:], 1.0)
                            rden = gp.tile([128, nt], FP32, tag="rden",
                                           bufs=2)
                            nc.vector.reciprocal(rden[:], den[:])
                            nc.vector.tensor_scalar(
                                out=tnew[:], in0=st_clo[:],
                                scalar1=cfg.ATGT,
                                scalar2=None, op0=ALU.subtract)
                            nc.vector.tensor_mul(tnew[:], tnew[:], rden[:])
                            wid = gp.tile([128, nt], FP32, tag="wid",
                                          bufs=2)
                            nc.vector.tensor_sub(wid[:], st_hi[:], st_lo[:])
                            nc.vector.tensor_mul(tnew[:], tnew[:], wid[:])
                            nc.vector.tensor_add(tnew[:], tnew[:], st_lo[:])
                        nc.vector.tensor_tensor(tnew[:], tnew[:], st_lo[:],
                                                ALU.max)
                        nc.vector.tensor_tensor(tnew[:], tnew[:], st_hi[:],
                                                ALU.min)
                        iseq = gp.tile([128, nt], U8, tag="iseq", bufs=2)
                        nc.vector.tensor_scalar(
                            out=iseq[:], in0=acc[:], scalar1=cfg.ATGT,
                            scalar2=None, op0=ALU.not_equal)
                        nc.vector.copy_predicated(st_t[:], iseq[:], tnew[:])

                    def s0():
                        nt0 = gp.tile([128, 1], FP32, tag="nt0")
                        nc.vector.memset(nt0[:], -0.1)
                        for nm, val in (("st_t", 0.1), ("st_lo", -2.1),
                                        ("st_hi", 2.1), ("st_clo", float(SS)),
                                        ("st_chi", float(-SS))):
                            t = gp.tile([128, nt], FP32, tag=nm, name=nm)
                            nc.vector.memset(t[:], val)
                            st[nm] = t
                        st["nt0"] = nt0
                        st["acc"] = gp.tile([128, nt], FP32, tag="acc",
                                            name="acc")
                        s16 = {}
                        for hi_, h in enumerate(heads):
                            for j in range(RC):
                                sp_ = ps512(f"selp_{hi_}_{j}")
                                nc.tensor.matmul(
                                    sp_[:],
                                    QnT[h][0:64, j * 128 : (j + 1) * 128],
                                    KnT[h][0:64, 1 : S : cfg.SUB],
                                    start=True, stop=True)
                                srow = gp.tile([128, SS], FP16,
                                               tag=f"s16_{hi_}_{j}",
                                               name=f"s16_{hi_}_{j}")
                                if (hi_ + j) % 2 == 0:
                                    nc.scalar.activation(srow[:], sp_[:],
                                                         AF.Copy)
                                else:
                                    nc.vector.tensor_copy(srow[:], sp_[:])
                                s16[(hi_, j)] = srow
                        st["s16"] = s16

                    def s1():  # it0 counts on ACT (Sign, acc space)
                        for hi_, h in enumerate(heads):
                            for j in range(RC):
                                col = hi_ * RC + j
                                scr = gp.tile([128, SS], FP8, tag="scr8",
                                              bufs=2, name="scr8")
                                nc.scalar.activation(
                                    scr[:], st["s16"][(hi_, j)][:], AF.Sign,
                                    bias=st["nt0"][:, 0:1],
                                    accum_out=st["acc"][:, col : col + 1])

                    def s2():  # it0 bracket + it1 counts on DVE
                        bracket_update(0)
                        for hi_, h in enumerate(heads):
                            for j in range(RC):
                                col = hi_ * RC + j
                                scr = gp.tile([128, SS], FP16,
                                              tag="scr16", bufs=2,
                                              name="scr16")
                                nc.vector.tensor_scalar(
                                    out=scr[:], in0=st["s16"][(hi_, j)][:],
                                    scalar1=st["st_t"][:, col : col + 1],
                                    scalar2=None, op0=ALU.is_ge,
                                    op1=ALU.add,
                                    accum_out=st["acc"][:, col : col + 1])
                        nc.vector.tensor_scalar(
                            out=st["acc"][:], in0=st["acc"][:], scalar1=2.0,
                            scalar2=float(-SS), op0=ALU.mult, op1=ALU.add)

                    def s3():  # final bracket + tneg -> QnT rows
                        bracket_update(1)
                        tneg = gp.tile([128, nt], FP16, tag="tneg")
                        nc.vector.tensor_scalar(
                            out=tneg[:], in0=st["st_t"][:], scalar1=-1.0,
                            scalar2=None, op0=ALU.mult)
                        ttp = ps.tile([nt, 128], FP16, tag="tps", bufs=2,
                                      padded_shape=[128, 512], name="ttp")
                        nc.tensor.transpose(ttp[:], tneg[:], ident[:])
                        tnT = gp.tile([nt, 128], FP16, tag="tnT")
                        nc.scalar.activation(tnT[:], ttp[:], AF.Copy)
                        for hi_, h in enumerate(heads):
                            for j in range(RC):
                                col = hi_ * RC + j
                                nc.sync.dma_start(
                                    QnT[h][64:65, j * 128 : (j + 1) * 128],
                                    tnT[col : col + 1, :])

                    return [s0, s1, s2, s3]

                def attention_heads(gi):
                    return [lambda h=h: attention_one(h)
                            for h in range(gi * G, (gi + 1) * G)]

                def attention_one(h):
                    if True:
                        avp = ps.tile([65, RS], FP32, tag="avp", bufs=2,
                                      padded_shape=[128, 512], name="avp")
                        for kcp in range(KC // 2):
                            em8 = pp.tile([128, 2, RS], FP8, tag="em8",
                                          bufs=4, name="em8")
                            stp = ps.tile([128, 2, RS], FP32, tag="pt",
                                          bufs=2,
                                          padded_shape=[128, 2, 512],
                                          name="stp")
                            for sub in range(2):
                                kc = 2 * kcp + sub
                                nc.tensor.matmul(
                                    stp[:, sub, :],
                                    KnT[h][:, kc * 128 : (kc + 1) * 128],
                                    QnT[h][:], start=True, stop=True)
                            e16 = pp.tile([128, 2, RS], FP16, tag="e16",
                                          bufs=2, name="e16")
                            nc.scalar.activation(e16[:], stp[:], AF.Exp)
                            nc.vector.scalar_tensor_tensor(
                                out=em8[:], in0=e16[:],
                                scalar=1.0, in1=e16[:],
                                op0=ALU.is_ge, op1=ALU.mult)
                            nc.tensor.matmul(
                                avp[:],
                                V8[:, 2 * kcp : 2 * kcp + 2, h, 0:65],
                                em8[:, :, :],
                                start=(kcp == 0), stop=(kcp == KC // 2 - 1),
                                perf_mode=DR)
                        # normalize: attnT = avp[0:64] * (1/z); z >= 1 by
                        # construction (the max score always passes t)
                        zrec = pp.tile([1, RS], FP32, tag="zrec", bufs=2,
                                       name="zrec")
                        nc.vector.reciprocal(zrec[:], avp[64:65, :])
                        zrep = pp.tile([64, RS], FP32, tag="zrep", bufs=2,
                                       name="zrep")
                        nc.gpsimd.partition_broadcast(zrep[:], zrec[:])
                        nc.vector.tensor_tensor(
                            attnT[(h % 2) * 64 : (h % 2) * 64 + 64,
                                  h // 2, :],
                            avp[0:64, :], zrep[:], ALU.mult)

                # ===== pipeline: selection(g) stages | attention(g-1) ====
                with (
                    tc.tile_pool(name="poolG0", bufs=1) as gp0,
                    tc.tile_pool(name="poolV", bufs=1) as pv,
                ):
                    stages0 = selection_stages(0, gp0)
                    vgen = proj_fp8(xt8, Wv, bias_t.get("bv"), KC, pv,
                                    "wv8")

                    def vchunks(n):
                        for _ in range(n):
                            j, pt = next(vgen)
                            dst = V8[:, j, :, 0:64]
                            src = pt[:].rearrange("p (h d) -> p h d", h=NH)
                            if j % 2 == 0:
                                nc.scalar.activation(dst, src, AF.Copy)
                            else:
                                nc.vector.tensor_copy(dst, src)

                    for s in stages0:
                        s()
                        vchunks(4)

            # poolX8 closed (xt8 freed)
            for gi in range(1, NG):
                with tc.tile_pool(name=f"poolG{gi}", bufs=1) as gp_:
                    stages = selection_stages(gi, gp_)
                    ah = attention_heads(gi - 1)
                    for s, a in zip(stages, ah):
                        s()
                        a()
            for a in attention_heads(NG - 1):
                a()

            # ================ phase C: out proj + gate ================
            with tc.tile_pool(name="poolC", bufs=1) as pc:
                wo_t = pc.tile([128, HP, D], FP16, tag="wo")
                nc.sync.dma_start(wo_t[:], Wo[:])
                xs_t = pc.tile([128, RC, D], FP32, tag="xs")
                nc.sync.dma_start(xs_t[:], xs[:])
                for j in range(RC):
                    op = pt1024("op_out")
                    for n in range(2):
                        sl = slice(n * 512, (n + 1) * 512)
                        for p in range(HP):
                            nc.tensor.matmul(
                                op[:, sl],
                                attnT[:, p, j * 128 : (j + 1) * 128],
                                wo_t[:, p, sl],
                                start=(p == 0),
                                stop=(p == HP - 1 and not with_bias))
                        if with_bias:
                            nc.tensor.matmul(
                                op[:, sl], ones_h[:], bias_t["bo"][:, sl],
                                start=False, stop=True)
                    dd = pc.tile([128, D], FP32, tag="dd", bufs=2,
                                 name="dd")
                    nc.vector.tensor_sub(dd[:], op[:], xs_t[:, j, :])
                    nc.vector.tensor_mul(dd[:], dd[:], gate16[:, j, :])
                    oo = pc.tile([128, D], FP32, tag="oo", bufs=2,
                                 name="oo")
                    nc.gpsimd.tensor_add(oo[:], dd[:], xs_t[:, j, :])
                    nc.sync.dma_start(out[:, j, :], oo[:])

    nc.finalize()
    return nc


# ---------------------------------------------------------------------------
_NC_CACHE = {}
LAST_EXEC_NS = None
LAST_RESULTS = None


def _get_nc(with_bias: bool):
    key = bool(with_bias)
    if key not in _NC_CACHE:
        _NC_CACHE[key] = build(Cfg(), key)
    return _NC_CACHE[key]


def _pack_core_inputs(x, Wq, bq, Wk, bk, Wv, bv, Wo, bo, Wt, bt, Wg, bg,
                      b, r0, cfg, with_bias, fp8):
    S, D, RS, DCH, HP = cfg.S, cfg.D, cfg.RS, cfg.DCH, cfg.HP
    xb = x[b]
    xt = np.ascontiguousarray(
        np.roll(xb.T, -r0, axis=1).reshape(DCH, 128, S).transpose(1, 0, 2))
    xss = np.ascontiguousarray(
        xb[r0 : r0 + RS].reshape(cfg.RC, 128, D).transpose(1, 0, 2))

    def wpack(W, dt, scale=1.0):
        return np.ascontiguousarray(
            (W * scale).reshape(DCH, 128, D).transpose(1, 0, 2)).astype(dt)

    m = {
        "x8T": xt.astype(fp8),
        "x16T": np.ascontiguousarray(xt[:, :, 0:RS]).astype(np.float16),
        "xs": xss.astype(np.float32),
        "Wq": wpack(Wq, fp8, WSCALE),
        "Wk": wpack(Wk, fp8, WSCALE),
        "Wv": wpack(Wv, fp8, WSCALE),
        "Wg": wpack(Wg, np.float16),
        "Wo": np.ascontiguousarray(
            (Wo / WSCALE).reshape(HP, 128, D).transpose(1, 0, 2))
            .astype(np.float16),
        "Wt": np.ascontiguousarray(
            Wt.reshape(DCH, 128).T * WSCALE).astype(fp8),
        "bt": bt.reshape(1, 1).astype(np.float32),
    }
    if with_bias:
        m["bq"] = (bq * WSCALE).reshape(1, D).astype(np.float16)
        m["bk"] = (bk * WSCALE).reshape(1, D).astype(np.float16)
        m["bv"] = (bv * WSCALE).reshape(1, D).astype(np.float16)
        m["bg"] = bg.reshape(1, D).astype(np.float16)
        m["bo"] = bo.reshape(1, D).astype(np.float16)
    return m


def kernel(**inputs):
    from concourse.bass_utils import run_bass_kernel_spmd
    cfg = Cfg()
    fp8 = mybir.dt.np(FP8)
    x = np.asarray(inputs["x"], np.float32)
    B, S, D = x.shape
    args = [np.asarray(inputs[k]) for k in
            ("Wq", "bq", "Wk", "bk", "Wv", "bv", "Wo", "bo", "Wt", "bt",
             "Wg", "bg")]
    with_bias = any(np.any(np.asarray(inputs[k])) for k in
                    ("bq", "bk", "bv", "bg", "bo"))
    nc = _get_nc(with_bias)
    in_maps = []
    for c in range(8):
        b, q = c // 4, c % 4
        in_maps.append(_pack_core_inputs(
            x, *args, b, q * cfg.RS, cfg, with_bias, fp8))
    trace = bool(int(os.environ.get("KERNEL_TRACE", "0")))
    res = run_bass_kernel_spmd(nc, in_maps, core_ids=list(range(8)),
                               trace=trace)
    global LAST_EXEC_NS, LAST_RESULTS
    LAST_EXEC_NS = res.exec_time_ns
    LAST_RESULTS = res
    out = np.empty((B, S, D), np.float32)
    for c in range(8):
        b, q = c // 4, c % 4
        o = res.results[c]["out"]  # [128, RC, D]
        out[b, q * cfg.RS : (q + 1) * cfg.RS] = \
            o.transpose(1, 0, 2).reshape(cfg.RS, D)
    return out
